# revision 31
# baseline (speedup 1.0000x reference)
"""Trainium2 Bass kernel for nn_ConvAttention (N=8, C=512, L=2048, 8 heads, causal).

Sharding: data-parallel over the batch dim N=8 -> one batch per NeuronCore.

v4 = v3 scheduling + fp8 DoubleRow on the PE-bound matmuls:
- Q/K projections contract two 128-channel tiles per matmul in fp8e4m3
  (Wq/Wk are pre-scaled by 64 on the host to stay in fp8 normal range; the
  softmax scale absorbs the 1/4096)
- off-diagonal attention*V matmuls contract two key tiles per matmul in fp8
  (exp output and V are quantized to fp8; every affected query attends to
  >=512 keys so the quantization noise averages out well below tolerance)
- V projection, Wo projection and the QK^T scores stay bf16 (early queries
  attend to few keys, so V-path fp8 noise would not average out there)
"""

import numpy as np
from contextlib import ExitStack

try:
    import concourse.bass as bass
except ImportError:  # concourse is on PYTHONPATH in the target container
    import sys
    sys.path.insert(0, "/opt/trn_rl_repo")
    import concourse.bass as bass

import concourse.tile as tile
from concourse import bacc, mybir
from concourse.bass_utils import run_bass_kernel_spmd

F32 = mybir.dt.float32
F32R = mybir.dt.float32r
BF16 = mybir.dt.bfloat16
FP8 = mybir.dt.float8e4
EXP = mybir.ActivationFunctionType.Exp
DR = mybir.MatmulPerfMode.DoubleRow

N_CORES = 8
N, C, L = 8, 512, 2048
H = 8
D = C // H            # 64
P = 128
CT = C // P           # 4 channel tiles
QBLK = 512            # q tile (matmul free dim)
NQT = L // QBLK       # 4 q tiles
HP = H // 2           # 4 head pairs (one per 128-channel tile)
W8_SCALE = 64.0       # host multiplies Wq/Wk by this before fp8 quantization
SCALE = float(C) ** -0.5 / (W8_SCALE * W8_SCALE)

BF_NAMES = ("wv", "wo")


def _emit(nc):
    # host passes x in bf16+fp8, Wv/Wo pre-transposed in bf16, Wq/Wk
    # pre-transposed, pre-scaled and quantized to fp8.
    x_d = nc.dram_tensor("x", [C, L], BF16, kind="ExternalInput").ap()
    x8_d = nc.dram_tensor("x8", [C, L], FP8, kind="ExternalInput").ap()
    wt_d = {nm: nc.dram_tensor(nm + "t", [C, C], BF16, kind="ExternalInput").ap()
            for nm in BF_NAMES}
    w8_d = {nm: nc.dram_tensor(nm + "t8", [C, C], FP8, kind="ExternalInput").ap()
            for nm in ("wq", "wk")}
    bo_d = nc.dram_tensor("bo", [C], F32, kind="ExternalInput").ap()
    y_d = nc.dram_tensor("y", [C, L], BF16, kind="ExternalOutput").ap()
    y_r = y_d.rearrange("(t p) l -> t p l", p=P)

    with tile.TileContext(nc) as tc, ExitStack() as ctx:
        const = ctx.enter_context(tc.tile_pool(name="const", bufs=1))
        persist = ctx.enter_context(tc.tile_pool(name="persist", bufs=1))

        ps_proj = ctx.enter_context(tc.tile_pool(name="ps_proj", bufs=2, space="PSUM"))
        ps_st = ctx.enter_context(tc.tile_pool(name="ps_st", bufs=2, space="PSUM"))
        ps_av = ctx.enter_context(tc.tile_pool(name="ps_av", bufs=2, space="PSUM"))

        # ---- warmup scratch (memset first so sim sees initialized data)
        warm_sb = const.tile([P, QBLK], BF16, tag="warm", name="warm_sb")
        nc.vector.memset(warm_sb, 0.0)
        scr_g = const.tile([P, 64], BF16, tag="scrg", name="scr_g")
        nc.gpsimd.memset(scr_g, 0.0)
        scr_e = const.tile([1, 16], F32, tag="scre", name="scr_e")

        # ---- persistent SBUF tensors
        wT = {nm: persist.tile([P, CT, C], BF16, tag=f"{nm}T", name=f"{nm}T")
              for nm in BF_NAMES}
        w8 = {nm: persist.tile([P, CT, C], FP8, tag=f"{nm}T8", name=f"{nm}T8")
              for nm in ("wq", "wk")}
        x_sb = persist.tile([P, CT, L], BF16, tag="x", name="x_sb")
        x8_sb = persist.tile([P, CT, L], FP8, tag="x8", name="x8_sb")
        k_sb = [persist.tile([P, L], BF16, tag=f"k{ot}", name=f"k{ot}")
                for ot in range(CT)]
        vt_sb = [persist.tile([P, H, D + 1], BF16, tag=f"vt{lt}", name=f"vt{lt}")
                 for lt in range(L // P)]
        # fp8 V for the off-diagonal kt pairs (kt < 12 only), interleaved by
        # kt parity for DoubleRow; 72-element stride keeps step%16==0
        vt8_sb = [persist.tile([P, 2, H, 72], FP8, tag=f"v8{pr}", name=f"v8{pr}")
                  for pr in range(6)]

        # Wo's ct3 lower-half rows, partition-shifted to 0-63 at startup so
        # the very last Wo matmuls can read the un-shifted head-b output
        # directly (no partition-shift DMA on the tail critical path)
        wo_b64 = persist.tile([D, C], BF16, tag="wob64", name="wo_b64")
        bo_sb = const.tile([P, CT], F32, tag="bo", name="bo_sb")
        onesH = const.tile([P, H], F32, tag="onesH", name="onesH")
        nc.vector.memset(onesH, 1.0)
        ones_f = const.tile([1, D], F32, tag="onesf", name="ones_f")
        nc.vector.memset(ones_f, 1.0)
        ones_r = const.tile([1, D], F32R, tag="onesr", name="ones_r")
        nc.vector.tensor_copy(ones_r, ones_f)

        # ---- input DMA: per-ct descriptors (each lands on its own hardware
        # DMA queue -> parallel transfers), posted from the three DMA-capable
        # engines. The ACT engine only gets a few early-critical ones (it is
        # the softmax pacemaker later); gpsimd posts a batch, then blocks on
        # its custom-op library load, then posts the non-urgent rest.
        wt_r = {nm: wt_d[nm].rearrange("(t p) o -> p t o", p=P) for nm in BF_NAMES}
        w8_r = {nm: w8_d[nm].rearrange("(t p) o -> p t o", p=P)
                for nm in ("wq", "wk")}
        x_r = x_d.rearrange("(t p) l -> p t l", p=P)
        x8_r = x8_d.rearrange("(t p) l -> p t l", p=P)

        def dreq(dst, src):
            return (dst, src)

        gp_early = [dreq(w8["wk"][:, ct, :], w8_r["wk"][:, ct, :])
                    for ct in (1, 2, 3)]
        gp_early += [dreq(x8_sb[:, ct, 0:QBLK], x8_r[:, ct, 0:QBLK])
                     for ct in (2, 3)]
        sc_early = [dreq(x8_sb[:, 1, 0:QBLK], x8_r[:, 1, 0:QBLK]),
                    dreq(w8["wq"][:, 1, :], w8_r["wq"][:, 1, :]),
                    dreq(wT["wv"][:, 1, :], wt_r["wv"][:, 1, :]),
                    dreq(wT["wv"][:, 3, :], wt_r["wv"][:, 3, :])]
        sy_all = [dreq(w8["wk"][:, 0, :], w8_r["wk"][:, 0, :]),
                  dreq(x8_sb[:, 0, 0:QBLK], x8_r[:, 0, 0:QBLK]),
                  dreq(w8["wq"][:, 0, :], w8_r["wq"][:, 0, :]),
                  dreq(w8["wq"][:, 2, :], w8_r["wq"][:, 2, :]),
                  dreq(w8["wq"][:, 3, :], w8_r["wq"][:, 3, :]),
                  dreq(wT["wv"][:, 0, :], wt_r["wv"][:, 0, :]),
                  dreq(wT["wv"][:, 2, :], wt_r["wv"][:, 2, :])]
        sy_all += [dreq(x_sb[:, ct, 0:QBLK], x_r[:, ct, 0:QBLK])
                   for ct in range(CT)]
        sy_all += [dreq(x8_sb[:, ct, QBLK:L], x8_r[:, ct, QBLK:L])
                   for ct in range(CT)]
        sy_all += [dreq(x_sb[:, ct, QBLK:L], x_r[:, ct, QBLK:L])
                   for ct in range(CT)]
        gp_late = [dreq(wT["wo"][:, ct, :], wt_r["wo"][:, ct, :])
                   for ct in range(CT)]
        gp_late.append(dreq(bo_sb, bo_d.rearrange("(t p) -> p t", p=P)))

        for dst, src in gp_early:
            nc.gpsimd.dma_start(dst, src)
        for dst, src in sc_early:
            nc.scalar.dma_start(dst, src)
        # gpsimd: force the custom-op library load now (affine_select +
        # partition_broadcast live in it; first use otherwise stalls ~8us)
        nc.gpsimd.affine_select(
            out=scr_g[:, 0:64], in_=scr_g[:, 0:64],
            compare_op=mybir.AluOpType.is_ge, fill=0.0,
            base=0, channel_multiplier=-1, pattern=[[1, 64]])
        nc.gpsimd.partition_broadcast(scr_g[:, 0:32], scr_g[0:1, 0:32])
        # scalar: pull the EXP table load forward
        nc.scalar.activation(scr_e, warm_sb[0:1, 0:16], EXP)
        for dst, src in sy_all:
            nc.sync.dma_start(dst, src)
        for dst, src in gp_late:
            nc.gpsimd.dma_start(dst, src)
        nc.gpsimd.dma_start(wo_b64, wT["wo"][D:P, 3, :])

        # tensor: ~12 throwaway matmuls get HAM past its 3.4us window so the
        # first real projections run at 2.4 GHz
        for i in range(12):
            wps = ps_proj.tile([P, QBLK], F32, tag="proj", name="warm_ps")
            nc.tensor.matmul(wps, lhsT=warm_sb[:, 0:P], rhs=warm_sb,
                             start=True, stop=True)

        q_pool = ctx.enter_context(tc.tile_pool(name="q", bufs=2))
        oc_pool = ctx.enter_context(tc.tile_pool(name="oc", bufs=2))
        pt_pool = ctx.enter_context(tc.tile_pool(name="pt", bufs=4))
        pt8_pool = ctx.enter_context(tc.tile_pool(name="pt8", bufs=3))
        nrm_pool = ctx.enter_context(tc.tile_pool(name="nrm", bufs=2))
        y_pool = ctx.enter_context(tc.tile_pool(name="y", bufs=2))

        # ---- projection helpers: each returns a list of unit thunks (one
        # PSUM group each) so filler work drips into the attention loop at
        # fine granularity. Q/K projections run fp8 DoubleRow (2 channel
        # tiles per matmul); V/Wo stay bf16.
        def qk_units(nm, ot, lc, fin_fn):
            def run():
                ps = ps_proj.tile([P, QBLK], F32, tag="proj", name="proj_ps")
                for cp in range(2):
                    nc.tensor.matmul(
                        ps, lhsT=w8[nm][:, 2 * cp:2 * cp + 2, ot * P:(ot + 1) * P],
                        rhs=x8_sb[:, 2 * cp:2 * cp + 2, lc * QBLK:(lc + 1) * QBLK],
                        start=(cp == 0), stop=(cp == 1), perf_mode=DR)
                fin_fn(ps)
            return [run]

        def proj_units(lhsT_of, rhs_of, fin_fn):
            def run():
                ps = ps_proj.tile([P, QBLK], F32, tag="proj", name="proj_ps")
                for ct in range(CT):
                    nc.tensor.matmul(
                        ps, lhsT=lhsT_of(ct), rhs=rhs_of(ct),
                        start=(ct == 0), stop=(ct == CT - 1))
                fin_fn(ps)
            return [run]

        def k_units(ot, lc):
            return qk_units(
                "wk", ot, lc,
                lambda ps: nc.vector.tensor_copy(
                    k_sb[ot][:, lc * QBLK:(lc + 1) * QBLK], ps))

        def v_units(lt):
            def fin(ps):
                t = vt_sb[lt]
                nc.vector.tensor_copy(t[:, :, D], onesH)
                nc.vector.tensor_copy(
                    t[:, :, 0:D], ps.rearrange("p (h d) -> p h d", d=D))
                if lt < 12:  # fp8 copy for the off-diagonal DoubleRow pairs
                    t8 = vt8_sb[lt // 2]
                    nc.vector.tensor_copy(t8[:, lt % 2, :, D], onesH)
                    nc.vector.tensor_copy(
                        t8[:, lt % 2, :, 0:D],
                        ps.rearrange("p (h d) -> p h d", d=D))
            return proj_units(
                lambda ct: x_sb[:, ct, lt * P:(lt + 1) * P],
                lambda ct: wT["wv"][:, ct, :], fin)

        q_tiles = {}

        def q_units(qt, ot):
            def fin(ps):
                nc.vector.tensor_copy(q_tiles[qt][:, ot, :], ps)
            units = qk_units("wq", ot, qt, fin)
            first = units[0]

            def f0():
                if qt not in q_tiles:
                    q_tiles[qt] = q_pool.tile([P, CT, QBLK], BF16, tag="q",
                                              name="q_sb")
                first()
            units[0] = f0
            return units

        oc_tiles = {}

        def wo_units(qt, ot):
            def fin(ps):
                ysb = y_pool.tile([P, QBLK], BF16, tag="y", name="y_sb")
                nc.vector.tensor_tensor(
                    ysb, ps, bo_sb[:, ot:ot + 1].to_broadcast((P, QBLK)),
                    mybir.AluOpType.add)
                nc.sync.dma_start(y_r[ot][:, qt * QBLK:(qt + 1) * QBLK], ysb)
            return proj_units(
                lambda ct: wT["wo"][:, ct, ot * P:(ot + 1) * P],
                lambda ct: oc_tiles[qt][ct], fin)

        def run_units(units):
            for u in units:
                u()

        # ---- warmup: the minimum for (qt0, hp0) to start
        run_units(k_units(0, 0))
        run_units(q_units(0, 0))
        run_units(v_units(0))

        # ---- filler queue: remaining projection work in need-order, drained
        # into the attention loop as PE filler.
        unit_q = []
        done = {("k", 0, 0), ("q", 0, 0), ("v", 0)}

        def enq(fid, units):
            for u in units[:-1]:
                unit_q.append((None, u))
            unit_q.append((fid, units[-1]))

        for lt in (1, 2, 3):
            enq(("v", lt), v_units(lt))
        for ot in (1, 2, 3):
            enq(("k", ot, 0), k_units(ot, 0))
            enq(("q", 0, ot), q_units(0, ot))
        for qt in (1, 2, 3):
            enq(("k", 0, qt), k_units(0, qt))
            enq(("q", qt, 0), q_units(qt, 0))
            for lt in range(4 * qt, 4 * qt + 4):
                enq(("v", lt), v_units(lt))
            for ot in (1, 2, 3):
                enq(("k", ot, qt), k_units(ot, qt))
                enq(("q", qt, ot), q_units(qt, ot))

        # Wo(qt-1) is reserved for qt's last head-pair so the PE stays at full
        # clock right up to the output tail
        wo_qs = {qt: [u for ot in range(CT) for u in wo_units(qt - 1, ot)]
                 for qt in (1, 2, 3)}

        def need(fid):
            if fid in done:
                return
            while unit_q:
                i, fn = unit_q.pop(0)
                fn()
                if i is not None:
                    done.add(i)
                    if i == fid:
                        return

        FILL_PER_KT = 0.4  # closures per kt (~340ns of PE work per kt slot)
        fill_acc = [0.0]

        def drip(qt, hp, kt, nkt):
            wq = wo_qs.get(qt) if hp == 3 else None
            if wq and kt % max(nkt // 4, 1) == 1:
                wq.pop(0)()
                return
            fill_acc[0] += FILL_PER_KT
            if unit_q and fill_acc[0] >= 1.0:
                fill_acc[0] -= 1.0
                i, fn = unit_q.pop(0)
                fn()
                if i is not None:
                    done.add(i)
            elif not unit_q and qt == NQT - 1 and hp >= 2 and kt % 3 == 0:
                # final stretch is ACT-bound; a sparse dummy matmul keeps the
                # HAM activity window alive so the tail runs at full clock
                dps = ps_proj.tile([P, QBLK], F32, tag="proj", name="ka_ps")
                nc.tensor.matmul(dps, lhsT=warm_sb[:, 0:P], rhs=warm_sb,
                                 start=True, stop=True, skip_group_check=True)

        # ---- attention
        pend_norm = [None]
        wo3_part = {}

        def run_pend_norm():
            if pend_norm[0] is not None:
                pend_norm[0]()
                pend_norm[0] = None

        for qt in range(NQT):
            oc_tiles[qt] = [oc_pool.tile([P, QBLK], BF16, tag=f"oc{j}",
                                         name=f"oc{j}") for j in range(CT)]
            oc = oc_tiles[qt]

            for hp in range(HP):
                need(("k", hp, qt))
                need(("q", qt, hp))
                q_sb = q_tiles[qt]
                nkt = 4 * qt + 4
                av = [ps_av.tile([65, QBLK], F32, tag="av", name="av_ps")
                      for _ in range(2)]
                pend_av = []
                cur8 = [None]
                for kt in range(nkt):
                    j = kt - 4 * qt          # >=0 -> diagonal block index
                    co = 0 if j < 0 else P * j
                    cols = QBLK - co
                    # head a's S^T in PSUM bank 0, head b's in bank 1 (two
                    # concurrent row-group matmuls must not share a bank)
                    stp = ps_st.tile([P, 2 * QBLK], F32, tag="st", name="st_ps")
                    for sub, ofs in ((0, 0), (1, QBLK)):
                        pofs = sub * D
                        nc.tensor.matmul(
                            stp[:, ofs:ofs + cols],
                            lhsT=k_sb[hp][pofs:pofs + D, kt * P:(kt + 1) * P],
                            rhs=q_sb[pofs:pofs + D, hp, co:QBLK],
                            start=True, stop=True)
                    sv = stp.rearrange("p (g c) -> p g c", c=QBLK)[:, :, 0:cols]
                    if j < 0:
                        # off-diagonal: exp straight to fp8, paired by kt
                        # parity for the DoubleRow AV
                        if kt % 2 == 0:
                            cur8[0] = pt8_pool.tile([P, 2, 2, QBLK], FP8,
                                                    tag="pt8", name="pt8")
                        pt8 = cur8[0]
                        nc.scalar.activation(pt8[:, kt % 2, :, :], sv, EXP,
                                             scale=SCALE)
                        if kt % 2 == 1:
                            def av_pair(pt8=pt8, pr=kt // 2):
                                need(("v", 2 * pr + 1))
                                for sub in range(2):
                                    nc.tensor.matmul(
                                        av[sub][:, 0:QBLK],
                                        lhsT=vt8_sb[pr][:, :, 2 * hp + sub,
                                                        0:D + 1],
                                        rhs=pt8[:, :, sub, :],
                                        start=(pr == 0), stop=True,
                                        skip_group_check=True, perf_mode=DR)
                            pend_av.append(av_pair)
                    else:
                        pt = pt_pool.tile([P, 2 * QBLK], BF16, tag="pt",
                                          name="pt_sb")
                        pv = pt.rearrange("p (g c) -> p g c",
                                          c=QBLK)[:, :, 0:cols]
                        nc.scalar.activation(pv, sv, EXP, scale=SCALE)
                        # only the first 128 columns of a diagonal tile touch
                        # the mask boundary; later columns are all-keep
                        for ofs in (0, QBLK):
                            sl = pt[:, ofs:ofs + P]
                            nc.gpsimd.affine_select(
                                out=sl, in_=sl,
                                compare_op=mybir.AluOpType.is_ge, fill=0.0,
                                base=0, channel_multiplier=-1,
                                pattern=[[1, P]])

                        def av_diag(pt=pt, kt=kt, co=co, cols=cols):
                            need(("v", kt))
                            for sub, ofs in ((0, 0), (1, QBLK)):
                                nc.tensor.matmul(
                                    av[sub][:, co:QBLK],
                                    lhsT=vt_sb[kt][:, 2 * hp + sub, :],
                                    rhs=pt[:, ofs:ofs + cols],
                                    start=(kt == 0), stop=True,
                                    skip_group_check=True)
                        pend_av.append(av_diag)
                    if kt == 1:
                        run_pend_norm()
                    drip(qt, hp, kt, nkt)
                    # pre-run Wo(qt3) ct0-2 for the first two output blocks
                    # during the final head-pair (held-open PSUM groups)
                    if qt == 3 and hp == 3 and kt in (nkt - 2, nkt - 1):
                        pot = kt - (nkt - 2)
                        wps = ps_proj.tile([P, QBLK], F32, tag="proj",
                                           name="proj_ps")
                        for ct in range(3):
                            nc.tensor.matmul(
                                wps,
                                lhsT=wT["wo"][:, ct, pot * P:(pot + 1) * P],
                                rhs=oc[ct], start=(ct == 0), stop=False,
                                skip_group_check=True)
                        wo3_part[pot] = wps
                    while len(pend_av) > 1:
                        pend_av.pop(0)()
                for fn in pend_av:
                    fn()

                last = (qt == NQT - 1 and hp == HP - 1)
                avs = nrm_pool.tile([65, 2, QBLK], F32, tag="avs", name="avs")
                if last:
                    # ---- tail norm, laid out for minimum critical path:
                    # pre-accumulate Wo ct0-2 for output blocks 2/3 in the
                    # retired score banks (blocks 0/1 are already pre-run in
                    # the proj banks), broadcast the raw denominator with
                    # small fp32r matmuls on the now-idle PE, then take the
                    # reciprocal across 64 lanes instead of 1; head b goes
                    # first so its partition-shift DMA overlaps head a's mul.
                    for pot in (2, 3):
                        wps = ps_st.tile([P, QBLK], F32, tag="st",
                                         name="wo_ps")
                        for ct in range(3):
                            nc.tensor.matmul(
                                wps,
                                lhsT=wT["wo"][:, ct, pot * P:(pot + 1) * P],
                                rhs=oc[ct], start=(ct == 0), stop=False,
                                skip_group_check=True)
                        wo3_part[pot] = wps
                    den0r = nrm_pool.tile([1, 2, QBLK], F32R, tag="den0r",
                                          name="den0r")
                    bc_ps = [ps_av.tile([D, QBLK], F32, tag="av", name="bc_ps")
                             for _ in range(2)]
                    inv = [nrm_pool.tile([D, QBLK], F32, tag=f"inv{s}",
                                         name="inv") for s in range(2)]
                    oab = [nrm_pool.tile([D, QBLK], BF16, tag=f"oab{s}",
                                         name="oab") for s in range(2)]
                    for sub in (1, 0):
                        nc.vector.tensor_copy(den0r[:, sub, :],
                                              av[sub][64:65, :])
                        nc.tensor.matmul(bc_ps[sub], lhsT=ones_r,
                                         rhs=den0r[:, sub, :],
                                         start=True, stop=True)
                        nc.vector.reciprocal_approx_fast(inv[sub], bc_ps[sub])
                        nc.vector.tensor_copy(avs[:, sub, :], av[sub])
                        nc.vector.tensor_mul(oab[sub], avs[0:D, sub, :],
                                             inv[sub])
                else:
                    # steady state: AV staging first frees the accumulators
                    # for the next head-pair ASAP
                    for sub in range(2):
                        nc.vector.tensor_copy(avs[:, sub, :], av[sub])
                    den0 = nrm_pool.tile([1, 2, QBLK], F32, tag="den0",
                                         name="den0")
                    nc.vector.tensor_copy(den0, avs[64:65, :, :])
                    nc.vector.reciprocal_approx_fast(den0, den0)

                    def norm_tail(hp=hp, avs=avs, den0=den0, oc=oc):
                        bc = nrm_pool.tile([D, 2, QBLK], F32, tag="bc",
                                           name="bc")
                        nc.gpsimd.partition_broadcast(bc, den0)
                        nc.vector.tensor_mul(
                            oc[hp][0:D, :], avs[0:D, 0, :], bc[:, 0, :])
                        tmp = nrm_pool.tile([D, QBLK], BF16, tag="tmp",
                                            name="tmp")
                        nc.vector.tensor_mul(tmp, avs[0:D, 1, :], bc[:, 1, :])
                        nc.sync.dma_start(oc[hp][D:P, :], tmp)
                    pend_norm[0] = norm_tail

            run_pend_norm()
            for u in wo_qs.pop(qt, []):
                u()

        # tail: drain leftover fillers and finish the last Wo projection; the
        # bias-add runs on the idle ACT engine (it reads PSUM directly)
        while unit_q:
            i, fn = unit_q.pop(0)
            fn()
        for ot in range(CT):
            wps = wo3_part[ot]
            nc.tensor.matmul(
                wps, lhsT=wT["wo"][0:D, 3, ot * P:(ot + 1) * P],
                rhs=oab[0], start=False, stop=False,
                skip_group_check=True)
            nc.tensor.matmul(
                wps, lhsT=wo_b64[:, ot * P:(ot + 1) * P],
                rhs=oab[1], start=False, stop=True,
                skip_group_check=True)
            ysb = y_pool.tile([P, QBLK], BF16, tag="y", name="y_sb")
            nc.scalar.add(ysb, wps, bo_sb[:, ot:ot + 1])
            nc.sync.dma_start(y_r[ot][:, 3 * QBLK:4 * QBLK], ysb)


_CACHE = {}


def _get_program():
    if "nc" not in _CACHE:
        nc = bacc.Bacc("TRN2", target_bir_lowering=False, debug=False,
                       num_devices=N_CORES)
        _emit(nc)
        nc.compile()
        _CACHE["nc"] = nc
    return _CACHE["nc"]


def _run(inputs, trace=False, **kwargs):
    import ml_dtypes
    nc = _get_program()
    bf16 = ml_dtypes.bfloat16
    fp8 = ml_dtypes.float8_e4m3
    xf = np.ascontiguousarray(np.asarray(inputs["x"], dtype=np.float32))
    x = xf.astype(bf16)
    x8 = xf.astype(fp8)
    shared = {}
    for nm in BF_NAMES:
        shared[nm + "t"] = np.ascontiguousarray(
            np.asarray(inputs[nm], dtype=np.float32).T).astype(bf16)
    for nm in ("wq", "wk"):
        shared[nm + "t8"] = np.ascontiguousarray(
            np.asarray(inputs[nm], dtype=np.float32).T * W8_SCALE).astype(fp8)
    shared["bo"] = np.ascontiguousarray(np.asarray(inputs["bo"], dtype=np.float32))
    in_maps = [{"x": np.ascontiguousarray(x[i]),
                "x8": np.ascontiguousarray(x8[i]), **shared}
               for i in range(N_CORES)]
    res = run_bass_kernel_spmd(nc, in_maps, core_ids=list(range(N_CORES)),
                               trace=trace, **kwargs)
    y = np.stack([np.asarray(res.results[i]["y"]).astype(np.float32)
                  for i in range(N_CORES)], axis=0)
    return y, res


def kernel(x, Wq, Wk, Wv, Wo, bo):
    y, _ = _run({"x": x, "wq": Wq, "wk": Wk, "wv": Wv, "wo": Wo, "bo": bo})
    return y


# revision 32
# speedup vs baseline: 1.0302x; 1.0302x over previous
"""Trainium2 Bass kernel for nn_ConvAttention (N=8, C=512, L=2048, 8 heads, causal).

Sharding: data-parallel over the batch dim N=8 -> one batch per NeuronCore.

v4 = v3 scheduling + fp8 DoubleRow on the PE-bound matmuls:
- Q/K projections contract two 128-channel tiles per matmul in fp8e4m3
  (Wq/Wk are pre-scaled by 64 on the host to stay in fp8 normal range; the
  softmax scale absorbs the 1/4096)
- off-diagonal attention*V matmuls contract two key tiles per matmul in fp8
  (exp output and V are quantized to fp8; every affected query attends to
  >=512 keys so the quantization noise averages out well below tolerance)
- V projection, Wo projection and the QK^T scores stay bf16 (early queries
  attend to few keys, so V-path fp8 noise would not average out there)
"""

import numpy as np
from contextlib import ExitStack

try:
    import concourse.bass as bass
except ImportError:  # concourse is on PYTHONPATH in the target container
    import sys
    sys.path.insert(0, "/opt/trn_rl_repo")
    import concourse.bass as bass

import concourse.tile as tile
from concourse import bacc, mybir
from concourse.bass_utils import run_bass_kernel_spmd

F32 = mybir.dt.float32
F32R = mybir.dt.float32r
BF16 = mybir.dt.bfloat16
FP8 = mybir.dt.float8e4
EXP = mybir.ActivationFunctionType.Exp
DR = mybir.MatmulPerfMode.DoubleRow

N_CORES = 8
N, C, L = 8, 512, 2048
H = 8
D = C // H            # 64
P = 128
CT = C // P           # 4 channel tiles
QBLK = 512            # q tile (matmul free dim)
NQT = L // QBLK       # 4 q tiles
HP = H // 2           # 4 head pairs (one per 128-channel tile)
W8_SCALE = 64.0       # host multiplies Wq/Wk by this before fp8 quantization
SCALE = float(C) ** -0.5 / (W8_SCALE * W8_SCALE)

BF_NAMES = ("wv", "wo")


def _emit(nc):
    # host passes x in bf16+fp8, Wv/Wo pre-transposed in bf16, Wq/Wk
    # pre-transposed, pre-scaled and quantized to fp8.
    x_d = nc.dram_tensor("x", [C, L], BF16, kind="ExternalInput").ap()
    x8_d = nc.dram_tensor("x8", [C, L], FP8, kind="ExternalInput").ap()
    wt_d = {nm: nc.dram_tensor(nm + "t", [C, C], BF16, kind="ExternalInput").ap()
            for nm in BF_NAMES}
    w8_d = {nm: nc.dram_tensor(nm + "t8", [C, C], FP8, kind="ExternalInput").ap()
            for nm in ("wq", "wk")}
    bo_d = nc.dram_tensor("bo", [C], F32, kind="ExternalInput").ap()
    y_d = nc.dram_tensor("y", [C, L], BF16, kind="ExternalOutput").ap()
    y_r = y_d.rearrange("(t p) l -> t p l", p=P)

    with tile.TileContext(nc) as tc, ExitStack() as ctx:
        const = ctx.enter_context(tc.tile_pool(name="const", bufs=1))
        persist = ctx.enter_context(tc.tile_pool(name="persist", bufs=1))

        ps_proj = ctx.enter_context(tc.tile_pool(name="ps_proj", bufs=2, space="PSUM"))
        ps_st = ctx.enter_context(tc.tile_pool(name="ps_st", bufs=2, space="PSUM"))
        ps_av = ctx.enter_context(tc.tile_pool(name="ps_av", bufs=2, space="PSUM"))

        # ---- warmup scratch (memset first so sim sees initialized data)
        warm_sb = const.tile([P, QBLK], BF16, tag="warm", name="warm_sb")
        nc.vector.memset(warm_sb, 0.0)
        scr_g = const.tile([P, 64], BF16, tag="scrg", name="scr_g")
        nc.gpsimd.memset(scr_g, 0.0)
        scr_e = const.tile([1, 16], F32, tag="scre", name="scr_e")

        # ---- persistent SBUF tensors
        wT = {nm: persist.tile([P, CT, C], BF16, tag=f"{nm}T", name=f"{nm}T")
              for nm in BF_NAMES}
        w8 = {nm: persist.tile([P, CT, C], FP8, tag=f"{nm}T8", name=f"{nm}T8")
              for nm in ("wq", "wk")}
        x_sb = persist.tile([P, CT, L], BF16, tag="x", name="x_sb")
        x8_sb = persist.tile([P, CT, L], FP8, tag="x8", name="x8_sb")
        k_sb = [persist.tile([P, L], BF16, tag=f"k{ot}", name=f"k{ot}")
                for ot in range(CT)]
        vt_sb = [persist.tile([P, H, D + 1], BF16, tag=f"vt{lt}", name=f"vt{lt}")
                 for lt in range(L // P)]
        # fp8 V for the off-diagonal kt pairs (kt < 12 only), interleaved by
        # kt parity for DoubleRow; 72-element stride keeps step%16==0
        vt8_sb = [persist.tile([P, 2, H, 72], FP8, tag=f"v8{pr}", name=f"v8{pr}")
                  for pr in range(6)]

        # Wo's ct3 lower-half rows, partition-shifted to 0-63 at startup so
        # the very last Wo matmuls can read the un-shifted head-b output
        # directly (no partition-shift DMA on the tail critical path)
        wo_b64 = persist.tile([D, C], BF16, tag="wob64", name="wo_b64")
        bo_sb = const.tile([P, CT], F32, tag="bo", name="bo_sb")
        onesH = const.tile([P, H], F32, tag="onesH", name="onesH")
        nc.vector.memset(onesH, 1.0)
        ones_f = const.tile([1, D], F32, tag="onesf", name="ones_f")
        nc.vector.memset(ones_f, 1.0)
        ones_r = const.tile([1, D], F32R, tag="onesr", name="ones_r")
        nc.vector.tensor_copy(ones_r, ones_f)

        # ---- input DMA: per-ct descriptors (each lands on its own hardware
        # DMA queue -> parallel transfers), posted from the three DMA-capable
        # engines. The ACT engine only gets a few early-critical ones (it is
        # the softmax pacemaker later); gpsimd posts a batch, then blocks on
        # its custom-op library load, then posts the non-urgent rest.
        wt_r = {nm: wt_d[nm].rearrange("(t p) o -> p t o", p=P) for nm in BF_NAMES}
        w8_r = {nm: w8_d[nm].rearrange("(t p) o -> p t o", p=P)
                for nm in ("wq", "wk")}
        x_r = x_d.rearrange("(t p) l -> p t l", p=P)
        x8_r = x8_d.rearrange("(t p) l -> p t l", p=P)

        def dreq(dst, src):
            return (dst, src)

        gp_early = [dreq(w8["wk"][:, ct, :], w8_r["wk"][:, ct, :])
                    for ct in (1, 2, 3)]
        gp_early += [dreq(x8_sb[:, ct, 0:QBLK], x8_r[:, ct, 0:QBLK])
                     for ct in (2, 3)]
        sc_early = [dreq(x8_sb[:, 1, 0:QBLK], x8_r[:, 1, 0:QBLK]),
                    dreq(w8["wq"][:, 1, :], w8_r["wq"][:, 1, :]),
                    dreq(wT["wv"][:, 1, :], wt_r["wv"][:, 1, :]),
                    dreq(wT["wv"][:, 3, :], wt_r["wv"][:, 3, :])]
        sy_all = [dreq(w8["wk"][:, 0, :], w8_r["wk"][:, 0, :]),
                  dreq(x8_sb[:, 0, 0:QBLK], x8_r[:, 0, 0:QBLK]),
                  dreq(w8["wq"][:, 0, :], w8_r["wq"][:, 0, :]),
                  dreq(w8["wq"][:, 2, :], w8_r["wq"][:, 2, :]),
                  dreq(w8["wq"][:, 3, :], w8_r["wq"][:, 3, :]),
                  dreq(wT["wv"][:, 0, :], wt_r["wv"][:, 0, :]),
                  dreq(wT["wv"][:, 2, :], wt_r["wv"][:, 2, :])]
        sy_all += [dreq(x_sb[:, ct, 0:QBLK], x_r[:, ct, 0:QBLK])
                   for ct in range(CT)]
        sy_all += [dreq(x8_sb[:, ct, QBLK:L], x8_r[:, ct, QBLK:L])
                   for ct in range(CT)]
        sy_all += [dreq(x_sb[:, ct, QBLK:L], x_r[:, ct, QBLK:L])
                   for ct in range(CT)]
        gp_late = [dreq(wT["wo"][:, ct, :], wt_r["wo"][:, ct, :])
                   for ct in range(CT)]
        gp_late.append(dreq(bo_sb, bo_d.rearrange("(t p) -> p t", p=P)))

        for dst, src in gp_early:
            nc.gpsimd.dma_start(dst, src)
        for dst, src in sc_early:
            nc.scalar.dma_start(dst, src)
        # gpsimd: force the custom-op library load now (affine_select +
        # partition_broadcast live in it; first use otherwise stalls ~8us)
        nc.gpsimd.affine_select(
            out=scr_g[:, 0:64], in_=scr_g[:, 0:64],
            compare_op=mybir.AluOpType.is_ge, fill=0.0,
            base=0, channel_multiplier=-1, pattern=[[1, 64]])
        nc.gpsimd.partition_broadcast(scr_g[:, 0:32], scr_g[0:1, 0:32])
        # scalar: pull the EXP table load forward
        nc.scalar.activation(scr_e, warm_sb[0:1, 0:16], EXP)
        for dst, src in sy_all:
            nc.sync.dma_start(dst, src)
        for dst, src in gp_late:
            nc.gpsimd.dma_start(dst, src)
        # partition-shift on sync: its queue is idle mid-kernel, so blocking
        # on the wo input landing costs nothing
        nc.sync.dma_start(wo_b64, wT["wo"][D:P, 3, :])

        # tensor: ~12 throwaway matmuls get HAM past its 3.4us window so the
        # first real projections run at 2.4 GHz
        for i in range(12):
            wps = ps_proj.tile([P, QBLK], F32, tag="proj", name="warm_ps")
            nc.tensor.matmul(wps, lhsT=warm_sb[:, 0:P], rhs=warm_sb,
                             start=True, stop=True)

        q_pool = ctx.enter_context(tc.tile_pool(name="q", bufs=2))
        oc_pool = ctx.enter_context(tc.tile_pool(name="oc", bufs=2))
        pt_pool = ctx.enter_context(tc.tile_pool(name="pt", bufs=4))
        pt8_pool = ctx.enter_context(tc.tile_pool(name="pt8", bufs=3))
        nrm_pool = ctx.enter_context(tc.tile_pool(name="nrm", bufs=2))
        y_pool = ctx.enter_context(tc.tile_pool(name="y", bufs=2))

        # ---- projection helpers: each returns a list of unit thunks (one
        # PSUM group each) so filler work drips into the attention loop at
        # fine granularity. Q/K projections run fp8 DoubleRow (2 channel
        # tiles per matmul); V/Wo stay bf16.
        def qk_units(nm, ot, lc, fin_fn):
            def run():
                ps = ps_proj.tile([P, QBLK], F32, tag="proj", name="proj_ps")
                for cp in range(2):
                    nc.tensor.matmul(
                        ps, lhsT=w8[nm][:, 2 * cp:2 * cp + 2, ot * P:(ot + 1) * P],
                        rhs=x8_sb[:, 2 * cp:2 * cp + 2, lc * QBLK:(lc + 1) * QBLK],
                        start=(cp == 0), stop=(cp == 1), perf_mode=DR)
                fin_fn(ps)
            return [run]

        def proj_units(lhsT_of, rhs_of, fin_fn):
            def run():
                ps = ps_proj.tile([P, QBLK], F32, tag="proj", name="proj_ps")
                for ct in range(CT):
                    nc.tensor.matmul(
                        ps, lhsT=lhsT_of(ct), rhs=rhs_of(ct),
                        start=(ct == 0), stop=(ct == CT - 1))
                fin_fn(ps)
            return [run]

        def k_units(ot, lc):
            return qk_units(
                "wk", ot, lc,
                lambda ps: nc.vector.tensor_copy(
                    k_sb[ot][:, lc * QBLK:(lc + 1) * QBLK], ps))

        def v_units(lt):
            def fin(ps):
                t = vt_sb[lt]
                nc.vector.tensor_copy(t[:, :, D], onesH)
                nc.vector.tensor_copy(
                    t[:, :, 0:D], ps.rearrange("p (h d) -> p h d", d=D))
                if lt < 12:  # fp8 copy for the off-diagonal DoubleRow pairs
                    t8 = vt8_sb[lt // 2]
                    nc.vector.tensor_copy(t8[:, lt % 2, :, D], onesH)
                    nc.vector.tensor_copy(
                        t8[:, lt % 2, :, 0:D],
                        ps.rearrange("p (h d) -> p h d", d=D))
            return proj_units(
                lambda ct: x_sb[:, ct, lt * P:(lt + 1) * P],
                lambda ct: wT["wv"][:, ct, :], fin)

        q_tiles = {}

        def q_units(qt, ot):
            def fin(ps):
                nc.vector.tensor_copy(q_tiles[qt][:, ot, :], ps)
            units = qk_units("wq", ot, qt, fin)
            first = units[0]

            def f0():
                if qt not in q_tiles:
                    q_tiles[qt] = q_pool.tile([P, CT, QBLK], BF16, tag="q",
                                              name="q_sb")
                first()
            units[0] = f0
            return units

        oc_tiles = {}

        def wo_units(qt, ot):
            def fin(ps):
                ysb = y_pool.tile([P, QBLK], BF16, tag="y", name="y_sb")
                nc.vector.tensor_tensor(
                    ysb, ps, bo_sb[:, ot:ot + 1].to_broadcast((P, QBLK)),
                    mybir.AluOpType.add)
                nc.sync.dma_start(y_r[ot][:, qt * QBLK:(qt + 1) * QBLK], ysb)
            return proj_units(
                lambda ct: wT["wo"][:, ct, ot * P:(ot + 1) * P],
                lambda ct: oc_tiles[qt][ct], fin)

        def run_units(units):
            for u in units:
                u()

        # ---- warmup: the minimum for (qt0, hp0) to start
        run_units(k_units(0, 0))
        run_units(q_units(0, 0))
        run_units(v_units(0))

        # ---- filler queue: remaining projection work in need-order, drained
        # into the attention loop as PE filler.
        unit_q = []
        done = {("k", 0, 0), ("q", 0, 0), ("v", 0)}

        def enq(fid, units):
            for u in units[:-1]:
                unit_q.append((None, u))
            unit_q.append((fid, units[-1]))

        for lt in (1, 2, 3):
            enq(("v", lt), v_units(lt))
        for ot in (1, 2, 3):
            enq(("k", ot, 0), k_units(ot, 0))
            enq(("q", 0, ot), q_units(0, ot))
        for qt in (1, 2, 3):
            enq(("k", 0, qt), k_units(0, qt))
            enq(("q", qt, 0), q_units(qt, 0))
            for lt in range(4 * qt, 4 * qt + 4):
                enq(("v", lt), v_units(lt))
            for ot in (1, 2, 3):
                enq(("k", ot, qt), k_units(ot, qt))
                enq(("q", qt, ot), q_units(qt, ot))

        # Wo(qt-1) is reserved for qt's last head-pair so the PE stays at full
        # clock right up to the output tail
        wo_qs = {qt: [u for ot in range(CT) for u in wo_units(qt - 1, ot)]
                 for qt in (1, 2, 3)}

        def need(fid):
            if fid in done:
                return
            while unit_q:
                i, fn = unit_q.pop(0)
                fn()
                if i is not None:
                    done.add(i)
                    if i == fid:
                        return

        FILL_PER_KT = 0.4  # closures per kt (~340ns of PE work per kt slot)
        fill_acc = [0.0]

        def drip(qt, hp, kt, nkt):
            wq = wo_qs.get(qt) if hp == 3 else None
            if wq and kt % max(nkt // 4, 1) == 1:
                wq.pop(0)()
                return
            fill_acc[0] += FILL_PER_KT
            if unit_q and fill_acc[0] >= 1.0:
                fill_acc[0] -= 1.0
                i, fn = unit_q.pop(0)
                fn()
                if i is not None:
                    done.add(i)
            elif not unit_q and qt == NQT - 1 and hp >= 2 and kt % 3 == 0:
                # final stretch is ACT-bound; a sparse dummy matmul keeps the
                # HAM activity window alive so the tail runs at full clock
                dps = ps_proj.tile([P, QBLK], F32, tag="proj", name="ka_ps")
                nc.tensor.matmul(dps, lhsT=warm_sb[:, 0:P], rhs=warm_sb,
                                 start=True, stop=True, skip_group_check=True)

        # ---- attention
        pend_norm = [None]
        wo3_part = {}

        def run_pend_norm():
            if pend_norm[0] is not None:
                pend_norm[0]()
                pend_norm[0] = None

        for qt in range(NQT):
            oc_tiles[qt] = [oc_pool.tile([P, QBLK], BF16, tag=f"oc{j}",
                                         name=f"oc{j}") for j in range(CT)]
            oc = oc_tiles[qt]

            for hp in range(HP):
                need(("k", hp, qt))
                need(("q", qt, hp))
                q_sb = q_tiles[qt]
                nkt = 4 * qt + 4
                av = [ps_av.tile([65, QBLK], F32, tag="av", name="av_ps")
                      for _ in range(2)]
                pend_av = []
                cur8 = [None]
                for kt in range(nkt):
                    j = kt - 4 * qt          # >=0 -> diagonal block index
                    co = 0 if j < 0 else P * j
                    cols = QBLK - co
                    # head a's S^T in PSUM bank 0, head b's in bank 1 (two
                    # concurrent row-group matmuls must not share a bank)
                    stp = ps_st.tile([P, 2 * QBLK], F32, tag="st", name="st_ps")
                    for sub, ofs in ((0, 0), (1, QBLK)):
                        pofs = sub * D
                        nc.tensor.matmul(
                            stp[:, ofs:ofs + cols],
                            lhsT=k_sb[hp][pofs:pofs + D, kt * P:(kt + 1) * P],
                            rhs=q_sb[pofs:pofs + D, hp, co:QBLK],
                            start=True, stop=True)
                    sv = stp.rearrange("p (g c) -> p g c", c=QBLK)[:, :, 0:cols]
                    if j < 0:
                        # off-diagonal: exp straight to fp8, paired by kt
                        # parity for the DoubleRow AV
                        if kt % 2 == 0:
                            cur8[0] = pt8_pool.tile([P, 2, 2, QBLK], FP8,
                                                    tag="pt8", name="pt8")
                        pt8 = cur8[0]
                        nc.scalar.activation(pt8[:, kt % 2, :, :], sv, EXP,
                                             scale=SCALE)
                        if kt % 2 == 1:
                            def av_pair(pt8=pt8, pr=kt // 2):
                                need(("v", 2 * pr + 1))
                                for sub in range(2):
                                    nc.tensor.matmul(
                                        av[sub][:, 0:QBLK],
                                        lhsT=vt8_sb[pr][:, :, 2 * hp + sub,
                                                        0:D + 1],
                                        rhs=pt8[:, :, sub, :],
                                        start=(pr == 0), stop=True,
                                        skip_group_check=True, perf_mode=DR)
                            pend_av.append(av_pair)
                    else:
                        pt = pt_pool.tile([P, 2 * QBLK], BF16, tag="pt",
                                          name="pt_sb")
                        pv = pt.rearrange("p (g c) -> p g c",
                                          c=QBLK)[:, :, 0:cols]
                        nc.scalar.activation(pv, sv, EXP, scale=SCALE)
                        # only the first 128 columns of a diagonal tile touch
                        # the mask boundary; later columns are all-keep
                        for ofs in (0, QBLK):
                            sl = pt[:, ofs:ofs + P]
                            nc.gpsimd.affine_select(
                                out=sl, in_=sl,
                                compare_op=mybir.AluOpType.is_ge, fill=0.0,
                                base=0, channel_multiplier=-1,
                                pattern=[[1, P]])

                        def av_diag(pt=pt, kt=kt, co=co, cols=cols):
                            need(("v", kt))
                            for sub, ofs in ((0, 0), (1, QBLK)):
                                nc.tensor.matmul(
                                    av[sub][:, co:QBLK],
                                    lhsT=vt_sb[kt][:, 2 * hp + sub, :],
                                    rhs=pt[:, ofs:ofs + cols],
                                    start=(kt == 0), stop=True,
                                    skip_group_check=True)
                        pend_av.append(av_diag)
                    if kt == 1:
                        run_pend_norm()
                    drip(qt, hp, kt, nkt)
                    # pre-run Wo(qt3) ct0-2 for the first two output blocks
                    # during the final head-pair (held-open PSUM groups)
                    if qt == 3 and hp == 3 and kt in (nkt - 2, nkt - 1):
                        pot = kt - (nkt - 2)
                        wps = ps_proj.tile([P, QBLK], F32, tag="proj",
                                           name="proj_ps")
                        for ct in range(3):
                            nc.tensor.matmul(
                                wps,
                                lhsT=wT["wo"][:, ct, pot * P:(pot + 1) * P],
                                rhs=oc[ct], start=(ct == 0), stop=False,
                                skip_group_check=True)
                        wo3_part[pot] = wps
                    while len(pend_av) > 1:
                        pend_av.pop(0)()
                for fn in pend_av:
                    fn()

                last = (qt == NQT - 1 and hp == HP - 1)
                avs = nrm_pool.tile([65, 2, QBLK], F32, tag="avs", name="avs")
                if last:
                    # ---- tail norm, laid out for minimum critical path:
                    # pre-accumulate Wo ct0-2 for output blocks 2/3 in the
                    # retired score banks (blocks 0/1 are already pre-run in
                    # the proj banks), broadcast the raw denominator with
                    # small fp32r matmuls on the now-idle PE, then take the
                    # reciprocal across 64 lanes instead of 1; head b goes
                    # first so its partition-shift DMA overlaps head a's mul.
                    for pot in (2, 3):
                        wps = ps_st.tile([P, QBLK], F32, tag="st",
                                         name="wo_ps")
                        for ct in range(3):
                            nc.tensor.matmul(
                                wps,
                                lhsT=wT["wo"][:, ct, pot * P:(pot + 1) * P],
                                rhs=oc[ct], start=(ct == 0), stop=False,
                                skip_group_check=True)
                        wo3_part[pot] = wps
                    den0r = nrm_pool.tile([1, 2, QBLK], F32R, tag="den0r",
                                          name="den0r")
                    bc_ps = [ps_av.tile([D, QBLK], F32, tag="av", name="bc_ps")
                             for _ in range(2)]
                    inv = [nrm_pool.tile([D, QBLK], F32, tag=f"inv{s}",
                                         name="inv") for s in range(2)]
                    oab = [nrm_pool.tile([D, QBLK], BF16, tag=f"oab{s}",
                                         name="oab") for s in range(2)]
                    for sub in (1, 0):
                        nc.vector.tensor_copy(den0r[:, sub, :],
                                              av[sub][64:65, :])
                        nc.tensor.matmul(bc_ps[sub], lhsT=ones_r,
                                         rhs=den0r[:, sub, :],
                                         start=True, stop=True)
                        nc.vector.reciprocal_approx_fast(inv[sub], bc_ps[sub])
                        nc.vector.tensor_copy(avs[:, sub, :], av[sub])
                        nc.vector.tensor_mul(oab[sub], avs[0:D, sub, :],
                                             inv[sub])
                else:
                    # steady state: AV staging first frees the accumulators
                    # for the next head-pair ASAP
                    for sub in range(2):
                        nc.vector.tensor_copy(avs[:, sub, :], av[sub])
                    den0 = nrm_pool.tile([1, 2, QBLK], F32, tag="den0",
                                         name="den0")
                    nc.vector.tensor_copy(den0, avs[64:65, :, :])
                    nc.vector.reciprocal_approx_fast(den0, den0)

                    def norm_tail(hp=hp, avs=avs, den0=den0, oc=oc):
                        bc = nrm_pool.tile([D, 2, QBLK], F32, tag="bc",
                                           name="bc")
                        nc.gpsimd.partition_broadcast(bc, den0)
                        nc.vector.tensor_mul(
                            oc[hp][0:D, :], avs[0:D, 0, :], bc[:, 0, :])
                        tmp = nrm_pool.tile([D, QBLK], BF16, tag="tmp",
                                            name="tmp")
                        nc.vector.tensor_mul(tmp, avs[0:D, 1, :], bc[:, 1, :])
                        nc.sync.dma_start(oc[hp][D:P, :], tmp)
                    pend_norm[0] = norm_tail

            run_pend_norm()
            for u in wo_qs.pop(qt, []):
                u()

        # tail: drain leftover fillers and finish the last Wo projection; the
        # bias-add runs on the idle ACT engine (it reads PSUM directly)
        while unit_q:
            i, fn = unit_q.pop(0)
            fn()
        for ot in range(CT):
            wps = wo3_part[ot]
            nc.tensor.matmul(
                wps, lhsT=wT["wo"][0:D, 3, ot * P:(ot + 1) * P],
                rhs=oab[0], start=False, stop=False,
                skip_group_check=True)
            nc.tensor.matmul(
                wps, lhsT=wo_b64[:, ot * P:(ot + 1) * P],
                rhs=oab[1], start=False, stop=True,
                skip_group_check=True)
            ysb = y_pool.tile([P, QBLK], BF16, tag="y", name="y_sb")
            nc.scalar.add(ysb, wps, bo_sb[:, ot:ot + 1])
            nc.sync.dma_start(y_r[ot][:, 3 * QBLK:4 * QBLK], ysb)


_CACHE = {}


def _get_program():
    if "nc" not in _CACHE:
        nc = bacc.Bacc("TRN2", target_bir_lowering=False, debug=False,
                       num_devices=N_CORES)
        _emit(nc)
        nc.compile()
        _CACHE["nc"] = nc
    return _CACHE["nc"]


def _run(inputs, trace=False, **kwargs):
    import ml_dtypes
    nc = _get_program()
    bf16 = ml_dtypes.bfloat16
    fp8 = ml_dtypes.float8_e4m3
    xf = np.ascontiguousarray(np.asarray(inputs["x"], dtype=np.float32))
    x = xf.astype(bf16)
    x8 = xf.astype(fp8)
    shared = {}
    for nm in BF_NAMES:
        shared[nm + "t"] = np.ascontiguousarray(
            np.asarray(inputs[nm], dtype=np.float32).T).astype(bf16)
    for nm in ("wq", "wk"):
        shared[nm + "t8"] = np.ascontiguousarray(
            np.asarray(inputs[nm], dtype=np.float32).T * W8_SCALE).astype(fp8)
    shared["bo"] = np.ascontiguousarray(np.asarray(inputs["bo"], dtype=np.float32))
    in_maps = [{"x": np.ascontiguousarray(x[i]),
                "x8": np.ascontiguousarray(x8[i]), **shared}
               for i in range(N_CORES)]
    res = run_bass_kernel_spmd(nc, in_maps, core_ids=list(range(N_CORES)),
                               trace=trace, **kwargs)
    y = np.stack([np.asarray(res.results[i]["y"]).astype(np.float32)
                  for i in range(N_CORES)], axis=0)
    return y, res


def kernel(x, Wq, Wk, Wv, Wo, bo):
    y, _ = _run({"x": x, "wq": Wq, "wk": Wk, "wv": Wv, "wo": Wo, "bo": bo})
    return y


# revision 33
# speedup vs baseline: 1.0318x; 1.0016x over previous
"""Trainium2 Bass kernel for nn_ConvAttention (N=8, C=512, L=2048, 8 heads, causal).

Sharding: data-parallel over the batch dim N=8 -> one batch per NeuronCore.

v4 = v3 scheduling + fp8 DoubleRow on the PE-bound matmuls:
- Q/K projections contract two 128-channel tiles per matmul in fp8e4m3
  (Wq/Wk are pre-scaled by 64 on the host to stay in fp8 normal range; the
  softmax scale absorbs the 1/4096)
- off-diagonal attention*V matmuls contract two key tiles per matmul in fp8
  (exp output and V are quantized to fp8; every affected query attends to
  >=512 keys so the quantization noise averages out well below tolerance)
- V projection, Wo projection and the QK^T scores stay bf16 (early queries
  attend to few keys, so V-path fp8 noise would not average out there)
"""

import numpy as np
from contextlib import ExitStack

try:
    import concourse.bass as bass
except ImportError:  # concourse is on PYTHONPATH in the target container
    import sys
    sys.path.insert(0, "/opt/trn_rl_repo")
    import concourse.bass as bass

import concourse.tile as tile
from concourse import bacc, mybir
from concourse.bass_utils import run_bass_kernel_spmd

F32 = mybir.dt.float32
F32R = mybir.dt.float32r
BF16 = mybir.dt.bfloat16
FP8 = mybir.dt.float8e4
EXP = mybir.ActivationFunctionType.Exp
DR = mybir.MatmulPerfMode.DoubleRow

N_CORES = 8
N, C, L = 8, 512, 2048
H = 8
D = C // H            # 64
P = 128
CT = C // P           # 4 channel tiles
QBLK = 512            # q tile (matmul free dim)
NQT = L // QBLK       # 4 q tiles
HP = H // 2           # 4 head pairs (one per 128-channel tile)
W8_SCALE = 64.0       # host multiplies Wq/Wk by this before fp8 quantization
SCALE = float(C) ** -0.5 / (W8_SCALE * W8_SCALE)

BF_NAMES = ("wv", "wo")


def _emit(nc):
    # host passes x in bf16+fp8, Wv/Wo pre-transposed in bf16, Wq/Wk
    # pre-transposed, pre-scaled and quantized to fp8.
    x_d = nc.dram_tensor("x", [C, L], BF16, kind="ExternalInput").ap()
    x8_d = nc.dram_tensor("x8", [C, L], FP8, kind="ExternalInput").ap()
    wt_d = {nm: nc.dram_tensor(nm + "t", [C, C], BF16, kind="ExternalInput").ap()
            for nm in BF_NAMES}
    w8_d = {nm: nc.dram_tensor(nm + "t8", [C, C], FP8, kind="ExternalInput").ap()
            for nm in ("wq", "wk")}
    bo_d = nc.dram_tensor("bo", [C], F32, kind="ExternalInput").ap()
    y_d = nc.dram_tensor("y", [C, L], BF16, kind="ExternalOutput").ap()
    y_r = y_d.rearrange("(t p) l -> t p l", p=P)

    with tile.TileContext(nc) as tc, ExitStack() as ctx:
        const = ctx.enter_context(tc.tile_pool(name="const", bufs=1))
        persist = ctx.enter_context(tc.tile_pool(name="persist", bufs=1))

        ps_proj = ctx.enter_context(tc.tile_pool(name="ps_proj", bufs=2, space="PSUM"))
        ps_st = ctx.enter_context(tc.tile_pool(name="ps_st", bufs=2, space="PSUM"))
        ps_av = ctx.enter_context(tc.tile_pool(name="ps_av", bufs=2, space="PSUM"))

        # ---- warmup scratch (memset first so sim sees initialized data)
        warm_sb = const.tile([P, QBLK], BF16, tag="warm", name="warm_sb")
        nc.vector.memset(warm_sb, 0.0)
        scr_g = const.tile([P, 64], BF16, tag="scrg", name="scr_g")
        nc.gpsimd.memset(scr_g, 0.0)
        scr_e = const.tile([1, 16], F32, tag="scre", name="scr_e")

        # ---- persistent SBUF tensors
        wT = {nm: persist.tile([P, CT, C], BF16, tag=f"{nm}T", name=f"{nm}T")
              for nm in BF_NAMES}
        w8 = {nm: persist.tile([P, CT, C], FP8, tag=f"{nm}T8", name=f"{nm}T8")
              for nm in ("wq", "wk")}
        x_sb = persist.tile([P, CT, L], BF16, tag="x", name="x_sb")
        x8_sb = persist.tile([P, CT, L], FP8, tag="x8", name="x8_sb")
        k_sb = [persist.tile([P, L], BF16, tag=f"k{ot}", name=f"k{ot}")
                for ot in range(CT)]
        vt_sb = [persist.tile([P, H, D + 1], BF16, tag=f"vt{lt}", name=f"vt{lt}")
                 for lt in range(L // P)]
        # fp8 V for the off-diagonal kt pairs (kt < 12 only), interleaved by
        # kt parity for DoubleRow; 72-element stride keeps step%16==0
        vt8_sb = [persist.tile([P, 2, H, 72], FP8, tag=f"v8{pr}", name=f"v8{pr}")
                  for pr in range(6)]

        bo_sb = const.tile([P, CT], F32, tag="bo", name="bo_sb")
        onesH = const.tile([P, H], F32, tag="onesH", name="onesH")
        nc.vector.memset(onesH, 1.0)
        ones_f = const.tile([1, D], F32, tag="onesf", name="ones_f")
        nc.vector.memset(ones_f, 1.0)
        ones_r = const.tile([1, D], F32R, tag="onesr", name="ones_r")
        nc.vector.tensor_copy(ones_r, ones_f)

        # ---- input DMA: per-ct descriptors (each lands on its own hardware
        # DMA queue -> parallel transfers), posted from the three DMA-capable
        # engines. The ACT engine only gets a few early-critical ones (it is
        # the softmax pacemaker later); gpsimd posts a batch, then blocks on
        # its custom-op library load, then posts the non-urgent rest.
        wt_r = {nm: wt_d[nm].rearrange("(t p) o -> p t o", p=P) for nm in BF_NAMES}
        w8_r = {nm: w8_d[nm].rearrange("(t p) o -> p t o", p=P)
                for nm in ("wq", "wk")}
        x_r = x_d.rearrange("(t p) l -> p t l", p=P)
        x8_r = x8_d.rearrange("(t p) l -> p t l", p=P)

        def dreq(dst, src):
            return (dst, src)

        gp_early = [dreq(w8["wk"][:, ct, :], w8_r["wk"][:, ct, :])
                    for ct in (1, 2, 3)]
        gp_early += [dreq(x8_sb[:, ct, 0:QBLK], x8_r[:, ct, 0:QBLK])
                     for ct in (2, 3)]
        sc_early = [dreq(x8_sb[:, 1, 0:QBLK], x8_r[:, 1, 0:QBLK]),
                    dreq(w8["wq"][:, 1, :], w8_r["wq"][:, 1, :]),
                    dreq(wT["wv"][:, 1, :], wt_r["wv"][:, 1, :]),
                    dreq(wT["wv"][:, 3, :], wt_r["wv"][:, 3, :])]
        sy_all = [dreq(w8["wk"][:, 0, :], w8_r["wk"][:, 0, :]),
                  dreq(x8_sb[:, 0, 0:QBLK], x8_r[:, 0, 0:QBLK]),
                  dreq(w8["wq"][:, 0, :], w8_r["wq"][:, 0, :]),
                  dreq(w8["wq"][:, 2, :], w8_r["wq"][:, 2, :]),
                  dreq(w8["wq"][:, 3, :], w8_r["wq"][:, 3, :]),
                  dreq(wT["wv"][:, 0, :], wt_r["wv"][:, 0, :]),
                  dreq(wT["wv"][:, 2, :], wt_r["wv"][:, 2, :])]
        sy_all += [dreq(x_sb[:, ct, 0:QBLK], x_r[:, ct, 0:QBLK])
                   for ct in range(CT)]
        sy_all += [dreq(x8_sb[:, ct, QBLK:L], x8_r[:, ct, QBLK:L])
                   for ct in range(CT)]
        sy_all += [dreq(x_sb[:, ct, QBLK:L], x_r[:, ct, QBLK:L])
                   for ct in range(CT)]
        gp_late = [dreq(wT["wo"][:, ct, :], wt_r["wo"][:, ct, :])
                   for ct in range(CT)]
        gp_late.append(dreq(bo_sb, bo_d.rearrange("(t p) -> p t", p=P)))

        for dst, src in gp_early:
            nc.gpsimd.dma_start(dst, src)
        for dst, src in sc_early:
            nc.scalar.dma_start(dst, src)
        # gpsimd: force the custom-op library load now (affine_select +
        # partition_broadcast live in it; first use otherwise stalls ~8us)
        nc.gpsimd.affine_select(
            out=scr_g[:, 0:64], in_=scr_g[:, 0:64],
            compare_op=mybir.AluOpType.is_ge, fill=0.0,
            base=0, channel_multiplier=-1, pattern=[[1, 64]])
        nc.gpsimd.partition_broadcast(scr_g[:, 0:32], scr_g[0:1, 0:32])
        # scalar: pull the EXP table load forward
        nc.scalar.activation(scr_e, warm_sb[0:1, 0:16], EXP)
        for dst, src in sy_all:
            nc.sync.dma_start(dst, src)
        for dst, src in gp_late:
            nc.gpsimd.dma_start(dst, src)

        # tensor: ~12 throwaway matmuls get HAM past its 3.4us window so the
        # first real projections run at 2.4 GHz
        for i in range(12):
            wps = ps_proj.tile([P, QBLK], F32, tag="proj", name="warm_ps")
            nc.tensor.matmul(wps, lhsT=warm_sb[:, 0:P], rhs=warm_sb,
                             start=True, stop=True)

        q_pool = ctx.enter_context(tc.tile_pool(name="q", bufs=2))
        oc_pool = ctx.enter_context(tc.tile_pool(name="oc", bufs=2))
        pt_pool = ctx.enter_context(tc.tile_pool(name="pt", bufs=4))
        pt8_pool = ctx.enter_context(tc.tile_pool(name="pt8", bufs=3))
        nrm_pool = ctx.enter_context(tc.tile_pool(name="nrm", bufs=2))
        y_pool = ctx.enter_context(tc.tile_pool(name="y", bufs=2))

        # ---- projection helpers: each returns a list of unit thunks (one
        # PSUM group each) so filler work drips into the attention loop at
        # fine granularity. Q/K projections run fp8 DoubleRow (2 channel
        # tiles per matmul); V/Wo stay bf16.
        def qk_units(nm, ot, lc, fin_fn):
            def run():
                ps = ps_proj.tile([P, QBLK], F32, tag="proj", name="proj_ps")
                for cp in range(2):
                    nc.tensor.matmul(
                        ps, lhsT=w8[nm][:, 2 * cp:2 * cp + 2, ot * P:(ot + 1) * P],
                        rhs=x8_sb[:, 2 * cp:2 * cp + 2, lc * QBLK:(lc + 1) * QBLK],
                        start=(cp == 0), stop=(cp == 1), perf_mode=DR)
                fin_fn(ps)
            return [run]

        def proj_units(lhsT_of, rhs_of, fin_fn):
            def run():
                ps = ps_proj.tile([P, QBLK], F32, tag="proj", name="proj_ps")
                for ct in range(CT):
                    nc.tensor.matmul(
                        ps, lhsT=lhsT_of(ct), rhs=rhs_of(ct),
                        start=(ct == 0), stop=(ct == CT - 1))
                fin_fn(ps)
            return [run]

        def k_units(ot, lc):
            return qk_units(
                "wk", ot, lc,
                lambda ps: nc.vector.tensor_copy(
                    k_sb[ot][:, lc * QBLK:(lc + 1) * QBLK], ps))

        def v_units(lt):
            def fin(ps):
                t = vt_sb[lt]
                nc.vector.tensor_copy(t[:, :, D], onesH)
                nc.vector.tensor_copy(
                    t[:, :, 0:D], ps.rearrange("p (h d) -> p h d", d=D))
                if lt < 12:  # fp8 copy for the off-diagonal DoubleRow pairs
                    t8 = vt8_sb[lt // 2]
                    nc.vector.tensor_copy(t8[:, lt % 2, :, D], onesH)
                    nc.vector.tensor_copy(
                        t8[:, lt % 2, :, 0:D],
                        ps.rearrange("p (h d) -> p h d", d=D))
            return proj_units(
                lambda ct: x_sb[:, ct, lt * P:(lt + 1) * P],
                lambda ct: wT["wv"][:, ct, :], fin)

        q_tiles = {}

        def q_units(qt, ot):
            def fin(ps):
                nc.vector.tensor_copy(q_tiles[qt][:, ot, :], ps)
            units = qk_units("wq", ot, qt, fin)
            first = units[0]

            def f0():
                if qt not in q_tiles:
                    q_tiles[qt] = q_pool.tile([P, CT, QBLK], BF16, tag="q",
                                              name="q_sb")
                first()
            units[0] = f0
            return units

        oc_tiles = {}

        def wo_units(qt, ot):
            def fin(ps):
                ysb = y_pool.tile([P, QBLK], BF16, tag="y", name="y_sb")
                nc.vector.tensor_tensor(
                    ysb, ps, bo_sb[:, ot:ot + 1].to_broadcast((P, QBLK)),
                    mybir.AluOpType.add)
                nc.sync.dma_start(y_r[ot][:, qt * QBLK:(qt + 1) * QBLK], ysb)
            return proj_units(
                lambda ct: wT["wo"][:, ct, ot * P:(ot + 1) * P],
                lambda ct: oc_tiles[qt][ct], fin)

        def run_units(units):
            for u in units:
                u()

        # ---- warmup: the minimum for (qt0, hp0) to start
        run_units(k_units(0, 0))
        run_units(q_units(0, 0))
        run_units(v_units(0))

        # ---- filler queue: remaining projection work in need-order, drained
        # into the attention loop as PE filler.
        unit_q = []
        done = {("k", 0, 0), ("q", 0, 0), ("v", 0)}

        def enq(fid, units):
            for u in units[:-1]:
                unit_q.append((None, u))
            unit_q.append((fid, units[-1]))

        for lt in (1, 2, 3):
            enq(("v", lt), v_units(lt))
        for ot in (1, 2, 3):
            enq(("k", ot, 0), k_units(ot, 0))
            enq(("q", 0, ot), q_units(0, ot))
        for qt in (1, 2, 3):
            enq(("k", 0, qt), k_units(0, qt))
            enq(("q", qt, 0), q_units(qt, 0))
            for lt in range(4 * qt, 4 * qt + 4):
                enq(("v", lt), v_units(lt))
            for ot in (1, 2, 3):
                enq(("k", ot, qt), k_units(ot, qt))
                enq(("q", qt, ot), q_units(qt, ot))

        # Wo(qt-1) is reserved for qt's last head-pair so the PE stays at full
        # clock right up to the output tail
        wo_qs = {qt: [u for ot in range(CT) for u in wo_units(qt - 1, ot)]
                 for qt in (1, 2, 3)}

        def need(fid):
            if fid in done:
                return
            while unit_q:
                i, fn = unit_q.pop(0)
                fn()
                if i is not None:
                    done.add(i)
                    if i == fid:
                        return

        FILL_PER_KT = 0.4  # closures per kt (~340ns of PE work per kt slot)
        fill_acc = [0.0]

        def drip(qt, hp, kt, nkt):
            wq = wo_qs.get(qt) if hp == 3 else None
            if wq and kt % max(nkt // 4, 1) == 1:
                wq.pop(0)()
                return
            fill_acc[0] += FILL_PER_KT
            if unit_q and fill_acc[0] >= 1.0:
                fill_acc[0] -= 1.0
                i, fn = unit_q.pop(0)
                fn()
                if i is not None:
                    done.add(i)
            elif not unit_q and qt == NQT - 1 and hp >= 2 and kt % 3 == 0:
                # final stretch is ACT-bound; a sparse dummy matmul keeps the
                # HAM activity window alive so the tail runs at full clock
                dps = ps_proj.tile([P, QBLK], F32, tag="proj", name="ka_ps")
                nc.tensor.matmul(dps, lhsT=warm_sb[:, 0:P], rhs=warm_sb,
                                 start=True, stop=True, skip_group_check=True)

        # ---- attention
        pend_norm = [None]
        wo3_part = {}

        def run_pend_norm():
            if pend_norm[0] is not None:
                pend_norm[0]()
                pend_norm[0] = None

        for qt in range(NQT):
            oc_tiles[qt] = [oc_pool.tile([P, QBLK], BF16, tag=f"oc{j}",
                                         name=f"oc{j}") for j in range(CT)]
            oc = oc_tiles[qt]

            for hp in range(HP):
                need(("k", hp, qt))
                need(("q", qt, hp))
                q_sb = q_tiles[qt]
                nkt = 4 * qt + 4
                av = [ps_av.tile([65, QBLK], F32, tag="av", name="av_ps")
                      for _ in range(2)]
                pend_av = []
                cur8 = [None]
                for kt in range(nkt):
                    j = kt - 4 * qt          # >=0 -> diagonal block index
                    co = 0 if j < 0 else P * j
                    cols = QBLK - co
                    # head a's S^T in PSUM bank 0, head b's in bank 1 (two
                    # concurrent row-group matmuls must not share a bank)
                    stp = ps_st.tile([P, 2 * QBLK], F32, tag="st", name="st_ps")
                    for sub, ofs in ((0, 0), (1, QBLK)):
                        pofs = sub * D
                        nc.tensor.matmul(
                            stp[:, ofs:ofs + cols],
                            lhsT=k_sb[hp][pofs:pofs + D, kt * P:(kt + 1) * P],
                            rhs=q_sb[pofs:pofs + D, hp, co:QBLK],
                            start=True, stop=True)
                    sv = stp.rearrange("p (g c) -> p g c", c=QBLK)[:, :, 0:cols]
                    if j < 0:
                        # off-diagonal: exp straight to fp8, paired by kt
                        # parity for the DoubleRow AV
                        if kt % 2 == 0:
                            cur8[0] = pt8_pool.tile([P, 2, 2, QBLK], FP8,
                                                    tag="pt8", name="pt8")
                        pt8 = cur8[0]
                        nc.scalar.activation(pt8[:, kt % 2, :, :], sv, EXP,
                                             scale=SCALE)
                        if kt % 2 == 1:
                            def av_pair(pt8=pt8, pr=kt // 2):
                                need(("v", 2 * pr + 1))
                                for sub in range(2):
                                    nc.tensor.matmul(
                                        av[sub][:, 0:QBLK],
                                        lhsT=vt8_sb[pr][:, :, 2 * hp + sub,
                                                        0:D + 1],
                                        rhs=pt8[:, :, sub, :],
                                        start=(pr == 0), stop=True,
                                        skip_group_check=True, perf_mode=DR)
                            pend_av.append(av_pair)
                    else:
                        pt = pt_pool.tile([P, 2 * QBLK], BF16, tag="pt",
                                          name="pt_sb")
                        pv = pt.rearrange("p (g c) -> p g c",
                                          c=QBLK)[:, :, 0:cols]
                        nc.scalar.activation(pv, sv, EXP, scale=SCALE)
                        # only the first 128 columns of a diagonal tile touch
                        # the mask boundary; later columns are all-keep
                        for ofs in (0, QBLK):
                            sl = pt[:, ofs:ofs + P]
                            nc.gpsimd.affine_select(
                                out=sl, in_=sl,
                                compare_op=mybir.AluOpType.is_ge, fill=0.0,
                                base=0, channel_multiplier=-1,
                                pattern=[[1, P]])

                        def av_diag(pt=pt, kt=kt, co=co, cols=cols):
                            need(("v", kt))
                            for sub, ofs in ((0, 0), (1, QBLK)):
                                nc.tensor.matmul(
                                    av[sub][:, co:QBLK],
                                    lhsT=vt_sb[kt][:, 2 * hp + sub, :],
                                    rhs=pt[:, ofs:ofs + cols],
                                    start=(kt == 0), stop=True,
                                    skip_group_check=True)
                        pend_av.append(av_diag)
                    if kt == 1:
                        run_pend_norm()
                    drip(qt, hp, kt, nkt)
                    # pre-run Wo(qt3) ct0-2 for the first two output blocks
                    # during the final head-pair (held-open PSUM groups)
                    if qt == 3 and hp == 3 and kt in (nkt - 2, nkt - 1):
                        pot = kt - (nkt - 2)
                        wps = ps_proj.tile([P, QBLK], F32, tag="proj",
                                           name="proj_ps")
                        for ct in range(3):
                            nc.tensor.matmul(
                                wps,
                                lhsT=wT["wo"][:, ct, pot * P:(pot + 1) * P],
                                rhs=oc[ct], start=(ct == 0), stop=False,
                                skip_group_check=True)
                        wo3_part[pot] = wps
                    while len(pend_av) > 1:
                        pend_av.pop(0)()
                for fn in pend_av:
                    fn()

                last = (qt == NQT - 1 and hp == HP - 1)
                avs = nrm_pool.tile([65, 2, QBLK], F32, tag="avs", name="avs")
                if last:
                    # ---- tail norm, laid out for minimum critical path:
                    # pre-accumulate Wo ct0-2 for output blocks 2/3 in the
                    # retired score banks (blocks 0/1 are already pre-run in
                    # the proj banks), broadcast the raw denominator with
                    # small fp32r matmuls on the now-idle PE, then take the
                    # reciprocal across 64 lanes instead of 1; head b goes
                    # first so its partition-shift DMA overlaps head a's mul.
                    for pot in (2, 3):
                        wps = ps_st.tile([P, QBLK], F32, tag="st",
                                         name="wo_ps")
                        for ct in range(3):
                            nc.tensor.matmul(
                                wps,
                                lhsT=wT["wo"][:, ct, pot * P:(pot + 1) * P],
                                rhs=oc[ct], start=(ct == 0), stop=False,
                                skip_group_check=True)
                        wo3_part[pot] = wps
                    # dummies (no deps) keep HAM warm while DVE works
                    for _ in range(14):
                        dps = ps_proj.tile([P, QBLK], F32, tag="proj",
                                           name="dummy_ps")
                        nc.tensor.matmul(dps, lhsT=warm_sb[:, 0:P],
                                         rhs=warm_sb, start=True, stop=True,
                                         skip_group_check=True)
                    den0r = nrm_pool.tile([1, 2, QBLK], F32R, tag="den0r",
                                          name="den0r")
                    bc_ps = [ps_av.tile([D, QBLK], F32, tag="av", name="bc_ps")
                             for _ in range(2)]
                    inv = [nrm_pool.tile([D, QBLK], F32, tag=f"inv{s}",
                                         name="inv") for s in range(2)]
                    for sub in (1, 0):
                        nc.vector.tensor_copy(den0r[:, sub, :],
                                              av[sub][64:65, :])
                        nc.tensor.matmul(bc_ps[sub], lhsT=ones_r,
                                         rhs=den0r[:, sub, :],
                                         start=True, stop=True)
                        nc.vector.reciprocal_approx_fast(inv[sub], bc_ps[sub])
                        nc.vector.tensor_copy(avs[:, sub, :], av[sub])
                        if sub == 1:
                            tmp = nrm_pool.tile([D, QBLK], BF16, tag="tmp",
                                                name="tmp")
                            nc.vector.tensor_mul(tmp, avs[0:D, 1, :], inv[1])
                            nc.sync.dma_start(oc[hp][D:P, :], tmp)
                        else:
                            nc.vector.tensor_mul(oc[hp][0:D, :],
                                                 avs[0:D, 0, :], inv[0])
                else:
                    # steady state: AV staging first frees the accumulators
                    # for the next head-pair ASAP
                    for sub in range(2):
                        nc.vector.tensor_copy(avs[:, sub, :], av[sub])
                    den0 = nrm_pool.tile([1, 2, QBLK], F32, tag="den0",
                                         name="den0")
                    nc.vector.tensor_copy(den0, avs[64:65, :, :])
                    nc.vector.reciprocal_approx_fast(den0, den0)

                    def norm_tail(hp=hp, avs=avs, den0=den0, oc=oc):
                        bc = nrm_pool.tile([D, 2, QBLK], F32, tag="bc",
                                           name="bc")
                        nc.gpsimd.partition_broadcast(bc, den0)
                        nc.vector.tensor_mul(
                            oc[hp][0:D, :], avs[0:D, 0, :], bc[:, 0, :])
                        tmp = nrm_pool.tile([D, QBLK], BF16, tag="tmp",
                                            name="tmp")
                        nc.vector.tensor_mul(tmp, avs[0:D, 1, :], bc[:, 1, :])
                        nc.sync.dma_start(oc[hp][D:P, :], tmp)
                    pend_norm[0] = norm_tail

            run_pend_norm()
            for u in wo_qs.pop(qt, []):
                u()

        # tail: drain leftover fillers and finish the last Wo projection; the
        # bias-add runs on the idle ACT engine (it reads PSUM directly)
        while unit_q:
            i, fn = unit_q.pop(0)
            fn()
        for ot in range(CT):
            wps = wo3_part[ot]
            nc.tensor.matmul(
                wps, lhsT=wT["wo"][:, 3, ot * P:(ot + 1) * P],
                rhs=oc_tiles[3][3], start=False, stop=True,
                skip_group_check=True)
            ysb = y_pool.tile([P, QBLK], BF16, tag="y", name="y_sb")
            nc.scalar.add(ysb, wps, bo_sb[:, ot:ot + 1])
            nc.sync.dma_start(y_r[ot][:, 3 * QBLK:4 * QBLK], ysb)


_CACHE = {}


def _get_program():
    if "nc" not in _CACHE:
        nc = bacc.Bacc("TRN2", target_bir_lowering=False, debug=False,
                       num_devices=N_CORES)
        _emit(nc)
        nc.compile()
        _CACHE["nc"] = nc
    return _CACHE["nc"]


def _run(inputs, trace=False, **kwargs):
    import ml_dtypes
    nc = _get_program()
    bf16 = ml_dtypes.bfloat16
    fp8 = ml_dtypes.float8_e4m3
    xf = np.ascontiguousarray(np.asarray(inputs["x"], dtype=np.float32))
    x = xf.astype(bf16)
    x8 = xf.astype(fp8)
    shared = {}
    for nm in BF_NAMES:
        shared[nm + "t"] = np.ascontiguousarray(
            np.asarray(inputs[nm], dtype=np.float32).T).astype(bf16)
    for nm in ("wq", "wk"):
        shared[nm + "t8"] = np.ascontiguousarray(
            np.asarray(inputs[nm], dtype=np.float32).T * W8_SCALE).astype(fp8)
    shared["bo"] = np.ascontiguousarray(np.asarray(inputs["bo"], dtype=np.float32))
    in_maps = [{"x": np.ascontiguousarray(x[i]),
                "x8": np.ascontiguousarray(x8[i]), **shared}
               for i in range(N_CORES)]
    res = run_bass_kernel_spmd(nc, in_maps, core_ids=list(range(N_CORES)),
                               trace=trace, **kwargs)
    y = np.stack([np.asarray(res.results[i]["y"]).astype(np.float32)
                  for i in range(N_CORES)], axis=0)
    return y, res


def kernel(x, Wq, Wk, Wv, Wo, bo):
    y, _ = _run({"x": x, "wq": Wq, "wk": Wk, "wv": Wv, "wo": Wo, "bo": bo})
    return y


# revision 34
# speedup vs baseline: 1.0377x; 1.0057x over previous
"""Trainium2 Bass kernel for nn_ConvAttention (N=8, C=512, L=2048, 8 heads, causal).

Sharding: data-parallel over the batch dim N=8 -> one batch per NeuronCore.

v4 = v3 scheduling + fp8 DoubleRow on the PE-bound matmuls:
- Q/K projections contract two 128-channel tiles per matmul in fp8e4m3
  (Wq/Wk are pre-scaled by 64 on the host to stay in fp8 normal range; the
  softmax scale absorbs the 1/4096)
- off-diagonal attention*V matmuls contract two key tiles per matmul in fp8
  (exp output and V are quantized to fp8; every affected query attends to
  >=512 keys so the quantization noise averages out well below tolerance)
- V projection, Wo projection and the QK^T scores stay bf16 (early queries
  attend to few keys, so V-path fp8 noise would not average out there)
"""

import numpy as np
from contextlib import ExitStack

try:
    import concourse.bass as bass
except ImportError:  # concourse is on PYTHONPATH in the target container
    import sys
    sys.path.insert(0, "/opt/trn_rl_repo")
    import concourse.bass as bass

import concourse.tile as tile
from concourse import bacc, mybir
from concourse.bass_utils import run_bass_kernel_spmd

F32 = mybir.dt.float32
F32R = mybir.dt.float32r
BF16 = mybir.dt.bfloat16
FP8 = mybir.dt.float8e4
EXP = mybir.ActivationFunctionType.Exp
DR = mybir.MatmulPerfMode.DoubleRow

N_CORES = 8
N, C, L = 8, 512, 2048
H = 8
D = C // H            # 64
P = 128
CT = C // P           # 4 channel tiles
QBLK = 512            # q tile (matmul free dim)
NQT = L // QBLK       # 4 q tiles
HP = H // 2           # 4 head pairs (one per 128-channel tile)
W8_SCALE = 64.0       # host multiplies Wq/Wk by this before fp8 quantization
SCALE = float(C) ** -0.5 / (W8_SCALE * W8_SCALE)

BF_NAMES = ("wv", "wo")


def _emit(nc):
    # host passes x in bf16+fp8, Wv/Wo pre-transposed in bf16, Wq/Wk
    # pre-transposed, pre-scaled and quantized to fp8.
    x_d = nc.dram_tensor("x", [C, L], BF16, kind="ExternalInput").ap()
    x8_d = nc.dram_tensor("x8", [C, L], FP8, kind="ExternalInput").ap()
    wt_d = {nm: nc.dram_tensor(nm + "t", [C, C], BF16, kind="ExternalInput").ap()
            for nm in BF_NAMES}
    w8_d = {nm: nc.dram_tensor(nm + "t8", [C, C], FP8, kind="ExternalInput").ap()
            for nm in ("wq", "wk")}
    bo_d = nc.dram_tensor("bo", [C], F32, kind="ExternalInput").ap()
    y_d = nc.dram_tensor("y", [C, L], BF16, kind="ExternalOutput").ap()
    y_r = y_d.rearrange("(t p) l -> t p l", p=P)

    with tile.TileContext(nc) as tc, ExitStack() as ctx:
        const = ctx.enter_context(tc.tile_pool(name="const", bufs=1))
        persist = ctx.enter_context(tc.tile_pool(name="persist", bufs=1))

        ps_proj = ctx.enter_context(tc.tile_pool(name="ps_proj", bufs=2, space="PSUM"))
        ps_st = ctx.enter_context(tc.tile_pool(name="ps_st", bufs=2, space="PSUM"))
        ps_av = ctx.enter_context(tc.tile_pool(name="ps_av", bufs=2, space="PSUM"))

        # ---- warmup scratch (memset first so sim sees initialized data)
        warm_sb = const.tile([P, QBLK], BF16, tag="warm", name="warm_sb")
        nc.vector.memset(warm_sb, 0.0)
        scr_g = const.tile([P, 64], BF16, tag="scrg", name="scr_g")
        nc.gpsimd.memset(scr_g, 0.0)
        scr_e = const.tile([1, 16], F32, tag="scre", name="scr_e")

        # ---- persistent SBUF tensors
        wT = {nm: persist.tile([P, CT, C], BF16, tag=f"{nm}T", name=f"{nm}T")
              for nm in BF_NAMES}
        w8 = {nm: persist.tile([P, CT, C], FP8, tag=f"{nm}T8", name=f"{nm}T8")
              for nm in ("wq", "wk")}
        x_sb = persist.tile([P, CT, L], BF16, tag="x", name="x_sb")
        x8_sb = persist.tile([P, CT, L], FP8, tag="x8", name="x8_sb")
        k_sb = [persist.tile([P, L], BF16, tag=f"k{ot}", name=f"k{ot}")
                for ot in range(CT)]
        vt_sb = [persist.tile([P, H, D + 1], BF16, tag=f"vt{lt}", name=f"vt{lt}")
                 for lt in range(L // P)]
        # fp8 V for the off-diagonal kt pairs (kt < 12 only), interleaved by
        # kt parity for DoubleRow; 72-element stride keeps step%16==0
        vt8_sb = [persist.tile([P, 2, H, 72], FP8, tag=f"v8{pr}", name=f"v8{pr}")
                  for pr in range(6)]

        bo_sb = const.tile([P, CT], F32, tag="bo", name="bo_sb")
        onesH = const.tile([P, H], F32, tag="onesH", name="onesH")
        nc.vector.memset(onesH, 1.0)
        ones_f = const.tile([1, D], F32, tag="onesf", name="ones_f")
        nc.vector.memset(ones_f, 1.0)
        ones_r = const.tile([1, D], F32R, tag="onesr", name="ones_r")
        nc.vector.tensor_copy(ones_r, ones_f)

        # ---- input DMA: per-ct descriptors (each lands on its own hardware
        # DMA queue -> parallel transfers), posted from the three DMA-capable
        # engines. The ACT engine only gets a few early-critical ones (it is
        # the softmax pacemaker later); gpsimd posts a batch, then blocks on
        # its custom-op library load, then posts the non-urgent rest.
        wt_r = {nm: wt_d[nm].rearrange("(t p) o -> p t o", p=P) for nm in BF_NAMES}
        w8_r = {nm: w8_d[nm].rearrange("(t p) o -> p t o", p=P)
                for nm in ("wq", "wk")}
        x_r = x_d.rearrange("(t p) l -> p t l", p=P)
        x8_r = x8_d.rearrange("(t p) l -> p t l", p=P)

        def dreq(dst, src):
            return (dst, src)

        gp_early = [dreq(w8["wk"][:, ct, :], w8_r["wk"][:, ct, :])
                    for ct in (1, 2, 3)]
        gp_early += [dreq(x8_sb[:, ct, 0:QBLK], x8_r[:, ct, 0:QBLK])
                     for ct in (2, 3)]
        sc_early = [dreq(x8_sb[:, 1, 0:QBLK], x8_r[:, 1, 0:QBLK]),
                    dreq(w8["wq"][:, 1, :], w8_r["wq"][:, 1, :]),
                    dreq(wT["wv"][:, 1, :], wt_r["wv"][:, 1, :]),
                    dreq(wT["wv"][:, 3, :], wt_r["wv"][:, 3, :])]
        sy_all = [dreq(w8["wk"][:, 0, :], w8_r["wk"][:, 0, :]),
                  dreq(x8_sb[:, 0, 0:QBLK], x8_r[:, 0, 0:QBLK]),
                  dreq(w8["wq"][:, 0, :], w8_r["wq"][:, 0, :]),
                  dreq(w8["wq"][:, 2, :], w8_r["wq"][:, 2, :]),
                  dreq(w8["wq"][:, 3, :], w8_r["wq"][:, 3, :]),
                  dreq(wT["wv"][:, 0, :], wt_r["wv"][:, 0, :]),
                  dreq(wT["wv"][:, 2, :], wt_r["wv"][:, 2, :])]
        sy_all += [dreq(x_sb[:, ct, 0:QBLK], x_r[:, ct, 0:QBLK])
                   for ct in range(CT)]
        sy_all += [dreq(x8_sb[:, ct, QBLK:L], x8_r[:, ct, QBLK:L])
                   for ct in range(CT)]
        sy_all += [dreq(x_sb[:, ct, QBLK:L], x_r[:, ct, QBLK:L])
                   for ct in range(CT)]
        gp_late = [dreq(wT["wo"][:, ct, :], wt_r["wo"][:, ct, :])
                   for ct in range(CT)]
        gp_late.append(dreq(bo_sb, bo_d.rearrange("(t p) -> p t", p=P)))

        for dst, src in gp_early:
            nc.gpsimd.dma_start(dst, src)
        for dst, src in sc_early:
            nc.scalar.dma_start(dst, src)
        # gpsimd: force the custom-op library load now (affine_select +
        # partition_broadcast live in it; first use otherwise stalls ~8us)
        nc.gpsimd.affine_select(
            out=scr_g[:, 0:64], in_=scr_g[:, 0:64],
            compare_op=mybir.AluOpType.is_ge, fill=0.0,
            base=0, channel_multiplier=-1, pattern=[[1, 64]])
        nc.gpsimd.partition_broadcast(scr_g[:, 0:32], scr_g[0:1, 0:32])
        # scalar: pull the EXP table load forward
        nc.scalar.activation(scr_e, warm_sb[0:1, 0:16], EXP)
        for dst, src in sy_all:
            nc.sync.dma_start(dst, src)
        for dst, src in gp_late:
            nc.gpsimd.dma_start(dst, src)

        # tensor: ~12 throwaway matmuls get HAM past its 3.4us window so the
        # first real projections run at 2.4 GHz
        for i in range(12):
            wps = ps_proj.tile([P, QBLK], F32, tag="proj", name="warm_ps")
            nc.tensor.matmul(wps, lhsT=warm_sb[:, 0:P], rhs=warm_sb,
                             start=True, stop=True)

        q_pool = ctx.enter_context(tc.tile_pool(name="q", bufs=2))
        oc_pool = ctx.enter_context(tc.tile_pool(name="oc", bufs=2))
        pt_pool = ctx.enter_context(tc.tile_pool(name="pt", bufs=4))
        pt8_pool = ctx.enter_context(tc.tile_pool(name="pt8", bufs=3))
        nrm_pool = ctx.enter_context(tc.tile_pool(name="nrm", bufs=2))
        y_pool = ctx.enter_context(tc.tile_pool(name="y", bufs=2))

        # ---- projection helpers: each returns a list of unit thunks (one
        # PSUM group each) so filler work drips into the attention loop at
        # fine granularity. Q/K projections run fp8 DoubleRow (2 channel
        # tiles per matmul); V/Wo stay bf16.
        def qk_units(nm, ot, lc, fin_fn):
            def run():
                ps = ps_proj.tile([P, QBLK], F32, tag="proj", name="proj_ps")
                for cp in range(2):
                    nc.tensor.matmul(
                        ps, lhsT=w8[nm][:, 2 * cp:2 * cp + 2, ot * P:(ot + 1) * P],
                        rhs=x8_sb[:, 2 * cp:2 * cp + 2, lc * QBLK:(lc + 1) * QBLK],
                        start=(cp == 0), stop=(cp == 1), perf_mode=DR)
                fin_fn(ps)
            return [run]

        def proj_units(lhsT_of, rhs_of, fin_fn):
            def run():
                ps = ps_proj.tile([P, QBLK], F32, tag="proj", name="proj_ps")
                for ct in range(CT):
                    nc.tensor.matmul(
                        ps, lhsT=lhsT_of(ct), rhs=rhs_of(ct),
                        start=(ct == 0), stop=(ct == CT - 1))
                fin_fn(ps)
            return [run]

        def k_units(ot, lc):
            return qk_units(
                "wk", ot, lc,
                lambda ps: nc.vector.tensor_copy(
                    k_sb[ot][:, lc * QBLK:(lc + 1) * QBLK], ps))

        def v_units(lt):
            def fin(ps):
                t = vt_sb[lt]
                nc.vector.tensor_copy(t[:, :, D], onesH)
                nc.vector.tensor_copy(
                    t[:, :, 0:D], ps.rearrange("p (h d) -> p h d", d=D))
                if lt < 12:  # fp8 copy for the off-diagonal DoubleRow pairs
                    t8 = vt8_sb[lt // 2]
                    nc.vector.tensor_copy(t8[:, lt % 2, :, D], onesH)
                    nc.vector.tensor_copy(
                        t8[:, lt % 2, :, 0:D],
                        ps.rearrange("p (h d) -> p h d", d=D))
            return proj_units(
                lambda ct: x_sb[:, ct, lt * P:(lt + 1) * P],
                lambda ct: wT["wv"][:, ct, :], fin)

        q_tiles = {}

        def q_units(qt, ot):
            def fin(ps):
                nc.vector.tensor_copy(q_tiles[qt][:, ot, :], ps)
            units = qk_units("wq", ot, qt, fin)
            first = units[0]

            def f0():
                if qt not in q_tiles:
                    q_tiles[qt] = q_pool.tile([P, CT, QBLK], BF16, tag="q",
                                              name="q_sb")
                first()
            units[0] = f0
            return units

        oc_tiles = {}

        def wo_units(qt, ot):
            def fin(ps):
                ysb = y_pool.tile([P, QBLK], BF16, tag="y", name="y_sb")
                nc.vector.tensor_tensor(
                    ysb, ps, bo_sb[:, ot:ot + 1].to_broadcast((P, QBLK)),
                    mybir.AluOpType.add)
                nc.sync.dma_start(y_r[ot][:, qt * QBLK:(qt + 1) * QBLK], ysb)
            return proj_units(
                lambda ct: wT["wo"][:, ct, ot * P:(ot + 1) * P],
                lambda ct: oc_tiles[qt][ct], fin)

        def run_units(units):
            for u in units:
                u()

        # ---- warmup: the minimum for (qt0, hp0) to start
        run_units(k_units(0, 0))
        run_units(q_units(0, 0))
        run_units(v_units(0))

        # ---- filler queue: remaining projection work in need-order, drained
        # into the attention loop as PE filler.
        unit_q = []
        done = {("k", 0, 0), ("q", 0, 0), ("v", 0)}

        def enq(fid, units):
            for u in units[:-1]:
                unit_q.append((None, u))
            unit_q.append((fid, units[-1]))

        for lt in (1, 2, 3):
            enq(("v", lt), v_units(lt))
        for ot in (1, 2, 3):
            enq(("k", ot, 0), k_units(ot, 0))
            enq(("q", 0, ot), q_units(0, ot))
        for qt in (1, 2, 3):
            enq(("k", 0, qt), k_units(0, qt))
            enq(("q", qt, 0), q_units(qt, 0))
            for lt in range(4 * qt, 4 * qt + 4):
                enq(("v", lt), v_units(lt))
            for ot in (1, 2, 3):
                enq(("k", ot, qt), k_units(ot, qt))
                enq(("q", qt, ot), q_units(qt, ot))

        # Wo(qt-1) is reserved for qt's last head-pair so the PE stays at full
        # clock right up to the output tail
        wo_qs = {qt: [u for ot in range(CT) for u in wo_units(qt - 1, ot)]
                 for qt in (1, 2, 3)}

        def need(fid):
            if fid in done:
                return
            while unit_q:
                i, fn = unit_q.pop(0)
                fn()
                if i is not None:
                    done.add(i)
                    if i == fid:
                        return

        FILL_PER_KT = 0.4  # closures per kt (~340ns of PE work per kt slot)
        fill_acc = [0.0]

        def drip(qt, hp, kt, nkt):
            wq = wo_qs.get(qt) if hp == 3 else None
            if wq and kt % max(nkt // 4, 1) == 1:
                wq.pop(0)()
                return
            fill_acc[0] += FILL_PER_KT
            if unit_q and fill_acc[0] >= 1.0:
                fill_acc[0] -= 1.0
                i, fn = unit_q.pop(0)
                fn()
                if i is not None:
                    done.add(i)


        # ---- attention
        pend_norm = [None]
        wo3_part = {}

        def run_pend_norm():
            if pend_norm[0] is not None:
                pend_norm[0]()
                pend_norm[0] = None

        for qt in range(NQT):
            oc_tiles[qt] = [oc_pool.tile([P, QBLK], BF16, tag=f"oc{j}",
                                         name=f"oc{j}") for j in range(CT)]
            oc = oc_tiles[qt]

            for hp in range(HP):
                need(("k", hp, qt))
                need(("q", qt, hp))
                q_sb = q_tiles[qt]
                nkt = 4 * qt + 4
                av = [ps_av.tile([65, QBLK], F32, tag="av", name="av_ps")
                      for _ in range(2)]
                pend_av = []
                cur8 = [None]
                for kt in range(nkt):
                    j = kt - 4 * qt          # >=0 -> diagonal block index
                    co = 0 if j < 0 else P * j
                    cols = QBLK - co
                    # head a's S^T in PSUM bank 0, head b's in bank 1 (two
                    # concurrent row-group matmuls must not share a bank)
                    stp = ps_st.tile([P, 2 * QBLK], F32, tag="st", name="st_ps")
                    for sub, ofs in ((0, 0), (1, QBLK)):
                        pofs = sub * D
                        nc.tensor.matmul(
                            stp[:, ofs:ofs + cols],
                            lhsT=k_sb[hp][pofs:pofs + D, kt * P:(kt + 1) * P],
                            rhs=q_sb[pofs:pofs + D, hp, co:QBLK],
                            start=True, stop=True)
                    sv = stp.rearrange("p (g c) -> p g c", c=QBLK)[:, :, 0:cols]
                    if j < 0:
                        # off-diagonal: exp straight to fp8, paired by kt
                        # parity for the DoubleRow AV
                        if kt % 2 == 0:
                            cur8[0] = pt8_pool.tile([P, 2, 2, QBLK], FP8,
                                                    tag="pt8", name="pt8")
                        pt8 = cur8[0]
                        nc.scalar.activation(pt8[:, kt % 2, :, :], sv, EXP,
                                             scale=SCALE)
                        if kt % 2 == 1:
                            def av_pair(pt8=pt8, pr=kt // 2):
                                need(("v", 2 * pr + 1))
                                for sub in range(2):
                                    nc.tensor.matmul(
                                        av[sub][:, 0:QBLK],
                                        lhsT=vt8_sb[pr][:, :, 2 * hp + sub,
                                                        0:D + 1],
                                        rhs=pt8[:, :, sub, :],
                                        start=(pr == 0), stop=True,
                                        skip_group_check=True, perf_mode=DR)
                            pend_av.append(av_pair)
                    else:
                        pt = pt_pool.tile([P, 2 * QBLK], BF16, tag="pt",
                                          name="pt_sb")
                        pv = pt.rearrange("p (g c) -> p g c",
                                          c=QBLK)[:, :, 0:cols]
                        nc.scalar.activation(pv, sv, EXP, scale=SCALE)
                        # only the first 128 columns of a diagonal tile touch
                        # the mask boundary (later columns are all-keep), and
                        # one 2D-pattern op masks both head slices at once
                        sl = pt.rearrange("p (g c) -> p g c",
                                          c=QBLK)[:, :, 0:P]
                        nc.gpsimd.affine_select(
                            out=sl, in_=sl,
                            compare_op=mybir.AluOpType.is_ge, fill=0.0,
                            base=0, channel_multiplier=-1,
                            pattern=[[0, 2], [1, P]])

                        def av_diag(pt=pt, kt=kt, co=co, cols=cols):
                            need(("v", kt))
                            for sub, ofs in ((0, 0), (1, QBLK)):
                                nc.tensor.matmul(
                                    av[sub][:, co:QBLK],
                                    lhsT=vt_sb[kt][:, 2 * hp + sub, :],
                                    rhs=pt[:, ofs:ofs + cols],
                                    start=(kt == 0), stop=True,
                                    skip_group_check=True)
                        pend_av.append(av_diag)
                    if kt == 1:
                        run_pend_norm()
                    drip(qt, hp, kt, nkt)
                    # pre-run Wo(qt3) ct0-2 for the first two output blocks
                    # during the final head-pair (held-open PSUM groups)
                    if qt == 3 and hp == 3 and kt in (nkt - 2, nkt - 1):
                        pot = kt - (nkt - 2)
                        wps = ps_proj.tile([P, QBLK], F32, tag="proj",
                                           name="proj_ps")
                        for ct in range(3):
                            nc.tensor.matmul(
                                wps,
                                lhsT=wT["wo"][:, ct, pot * P:(pot + 1) * P],
                                rhs=oc[ct], start=(ct == 0), stop=False,
                                skip_group_check=True)
                        wo3_part[pot] = wps
                    while len(pend_av) > 1:
                        pend_av.pop(0)()
                for fn in pend_av:
                    fn()

                last = (qt == NQT - 1 and hp == HP - 1)
                avs = nrm_pool.tile([65, 2, QBLK], F32, tag="avs", name="avs")
                if last:
                    # ---- tail norm, laid out for minimum critical path:
                    # pre-accumulate Wo ct0-2 for output blocks 2/3 in the
                    # retired score banks (blocks 0/1 are already pre-run in
                    # the proj banks), broadcast the raw denominator with
                    # small fp32r matmuls on the now-idle PE, then take the
                    # reciprocal across 64 lanes instead of 1; head b goes
                    # first so its partition-shift DMA overlaps head a's mul.
                    for pot in (2, 3):
                        wps = ps_st.tile([P, QBLK], F32, tag="st",
                                         name="wo_ps")
                        for ct in range(3):
                            nc.tensor.matmul(
                                wps,
                                lhsT=wT["wo"][:, ct, pot * P:(pot + 1) * P],
                                rhs=oc[ct], start=(ct == 0), stop=False,
                                skip_group_check=True)
                        wo3_part[pot] = wps
                    den0r = nrm_pool.tile([1, 2, QBLK], F32R, tag="den0r",
                                          name="den0r")
                    bc_ps = [ps_av.tile([D, QBLK], F32, tag="av", name="bc_ps")
                             for _ in range(2)]
                    inv = [nrm_pool.tile([D, QBLK], F32, tag=f"inv{s}",
                                         name="inv") for s in range(2)]
                    for sub in (1, 0):
                        nc.vector.tensor_copy(den0r[:, sub, :],
                                              av[sub][64:65, :])
                        nc.tensor.matmul(bc_ps[sub], lhsT=ones_r,
                                         rhs=den0r[:, sub, :],
                                         start=True, stop=True)
                        nc.vector.reciprocal_approx_fast(inv[sub], bc_ps[sub])
                        nc.vector.tensor_copy(avs[:, sub, :], av[sub])
                        if sub == 1:
                            tmp = nrm_pool.tile([D, QBLK], BF16, tag="tmp",
                                                name="tmp")
                            nc.vector.tensor_mul(tmp, avs[0:D, 1, :], inv[1])
                            nc.sync.dma_start(oc[hp][D:P, :], tmp)
                        else:
                            nc.vector.tensor_mul(oc[hp][0:D, :],
                                                 avs[0:D, 0, :], inv[0])
                else:
                    # steady state: AV staging first frees the accumulators
                    # for the next head-pair ASAP
                    for sub in range(2):
                        nc.vector.tensor_copy(avs[:, sub, :], av[sub])
                    den0 = nrm_pool.tile([1, 2, QBLK], F32, tag="den0",
                                         name="den0")
                    nc.vector.tensor_copy(den0, avs[64:65, :, :])
                    nc.vector.reciprocal_approx_fast(den0, den0)

                    def norm_tail(hp=hp, avs=avs, den0=den0, oc=oc):
                        bc = nrm_pool.tile([D, 2, QBLK], F32, tag="bc",
                                           name="bc")
                        nc.gpsimd.partition_broadcast(bc, den0)
                        nc.vector.tensor_mul(
                            oc[hp][0:D, :], avs[0:D, 0, :], bc[:, 0, :])
                        tmp = nrm_pool.tile([D, QBLK], BF16, tag="tmp",
                                            name="tmp")
                        nc.vector.tensor_mul(tmp, avs[0:D, 1, :], bc[:, 1, :])
                        nc.sync.dma_start(oc[hp][D:P, :], tmp)
                    pend_norm[0] = norm_tail

            run_pend_norm()
            for u in wo_qs.pop(qt, []):
                u()

        # tail: drain leftover fillers and finish the last Wo projection; the
        # bias-add runs on the idle ACT engine (it reads PSUM directly)
        while unit_q:
            i, fn = unit_q.pop(0)
            fn()
        for ot in range(CT):
            wps = wo3_part[ot]
            nc.tensor.matmul(
                wps, lhsT=wT["wo"][:, 3, ot * P:(ot + 1) * P],
                rhs=oc_tiles[3][3], start=False, stop=True,
                skip_group_check=True)
            ysb = y_pool.tile([P, QBLK], BF16, tag="y", name="y_sb")
            nc.scalar.add(ysb, wps, bo_sb[:, ot:ot + 1])
            nc.sync.dma_start(y_r[ot][:, 3 * QBLK:4 * QBLK], ysb)


_CACHE = {}


def _get_program():
    if "nc" not in _CACHE:
        nc = bacc.Bacc("TRN2", target_bir_lowering=False, debug=False,
                       num_devices=N_CORES)
        _emit(nc)
        nc.compile()
        _CACHE["nc"] = nc
    return _CACHE["nc"]


def _run(inputs, trace=False, **kwargs):
    import ml_dtypes
    nc = _get_program()
    bf16 = ml_dtypes.bfloat16
    fp8 = ml_dtypes.float8_e4m3
    xf = np.ascontiguousarray(np.asarray(inputs["x"], dtype=np.float32))
    x = xf.astype(bf16)
    x8 = xf.astype(fp8)
    shared = {}
    for nm in BF_NAMES:
        shared[nm + "t"] = np.ascontiguousarray(
            np.asarray(inputs[nm], dtype=np.float32).T).astype(bf16)
    for nm in ("wq", "wk"):
        shared[nm + "t8"] = np.ascontiguousarray(
            np.asarray(inputs[nm], dtype=np.float32).T * W8_SCALE).astype(fp8)
    shared["bo"] = np.ascontiguousarray(np.asarray(inputs["bo"], dtype=np.float32))
    in_maps = [{"x": np.ascontiguousarray(x[i]),
                "x8": np.ascontiguousarray(x8[i]), **shared}
               for i in range(N_CORES)]
    res = run_bass_kernel_spmd(nc, in_maps, core_ids=list(range(N_CORES)),
                               trace=trace, **kwargs)
    y = np.stack([np.asarray(res.results[i]["y"]).astype(np.float32)
                  for i in range(N_CORES)], axis=0)
    return y, res


def kernel(x, Wq, Wk, Wv, Wo, bo):
    y, _ = _run({"x": x, "wq": Wq, "wk": Wk, "wv": Wv, "wo": Wo, "bo": bo})
    return y


# revision 35
# speedup vs baseline: 1.0571x; 1.0187x over previous
"""Trainium2 Bass kernel for nn_ConvAttention (N=8, C=512, L=2048, 8 heads, causal).

Sharding: data-parallel over the batch dim N=8 -> one batch per NeuronCore.

v4 = v3 scheduling + fp8 DoubleRow on the PE-bound matmuls:
- Q/K projections contract two 128-channel tiles per matmul in fp8e4m3
  (Wq/Wk are pre-scaled by 64 on the host to stay in fp8 normal range; the
  softmax scale absorbs the 1/4096)
- off-diagonal attention*V matmuls contract two key tiles per matmul in fp8
  (exp output and V are quantized to fp8; every affected query attends to
  >=512 keys so the quantization noise averages out well below tolerance)
- V projection, Wo projection and the QK^T scores stay bf16 (early queries
  attend to few keys, so V-path fp8 noise would not average out there)
"""

import numpy as np
from contextlib import ExitStack

try:
    import concourse.bass as bass
except ImportError:  # concourse is on PYTHONPATH in the target container
    import sys
    sys.path.insert(0, "/opt/trn_rl_repo")
    import concourse.bass as bass

import concourse.tile as tile
from concourse import bacc, mybir
from concourse.bass_utils import run_bass_kernel_spmd

F32 = mybir.dt.float32
F32R = mybir.dt.float32r
BF16 = mybir.dt.bfloat16
FP8 = mybir.dt.float8e4
EXP = mybir.ActivationFunctionType.Exp
DR = mybir.MatmulPerfMode.DoubleRow

N_CORES = 8
N, C, L = 8, 512, 2048
H = 8
D = C // H            # 64
P = 128
CT = C // P           # 4 channel tiles
QBLK = 512            # q tile (matmul free dim)
NQT = L // QBLK       # 4 q tiles
HP = H // 2           # 4 head pairs (one per 128-channel tile)
W8_SCALE = 64.0       # host multiplies Wq/Wk by this before fp8 quantization
SCALE = float(C) ** -0.5 / (W8_SCALE * W8_SCALE)

BF_NAMES = ("wv", "wo")


def _emit(nc):
    # host passes x in bf16+fp8, Wv/Wo pre-transposed in bf16, Wq/Wk
    # pre-transposed, pre-scaled and quantized to fp8.
    x_d = nc.dram_tensor("x", [C, L], BF16, kind="ExternalInput").ap()
    x8_d = nc.dram_tensor("x8", [C, L], FP8, kind="ExternalInput").ap()
    wt_d = {nm: nc.dram_tensor(nm + "t", [C, C], BF16, kind="ExternalInput").ap()
            for nm in BF_NAMES}
    w8_d = {nm: nc.dram_tensor(nm + "t8", [C, C], FP8, kind="ExternalInput").ap()
            for nm in ("wq", "wk")}
    bo_d = nc.dram_tensor("bo", [C], F32, kind="ExternalInput").ap()
    y_d = nc.dram_tensor("y", [C, L], BF16, kind="ExternalOutput").ap()
    y_r = y_d.rearrange("(t p) l -> t p l", p=P)

    with tile.TileContext(nc) as tc, ExitStack() as ctx:
        const = ctx.enter_context(tc.tile_pool(name="const", bufs=1))
        persist = ctx.enter_context(tc.tile_pool(name="persist", bufs=1))

        ps_proj = ctx.enter_context(tc.tile_pool(name="ps_proj", bufs=2, space="PSUM"))
        ps_st = ctx.enter_context(tc.tile_pool(name="ps_st", bufs=2, space="PSUM"))
        ps_av = ctx.enter_context(tc.tile_pool(name="ps_av", bufs=2, space="PSUM"))

        # ---- warmup scratch (memset first so sim sees initialized data)
        warm_sb = const.tile([P, QBLK], BF16, tag="warm", name="warm_sb")
        nc.vector.memset(warm_sb, 0.0)
        scr_g = const.tile([P, 64], BF16, tag="scrg", name="scr_g")
        nc.gpsimd.memset(scr_g, 0.0)
        scr_e = const.tile([1, 16], F32, tag="scre", name="scr_e")

        # ---- persistent SBUF tensors
        wT = {nm: persist.tile([P, CT, C], BF16, tag=f"{nm}T", name=f"{nm}T")
              for nm in BF_NAMES}
        w8 = {nm: persist.tile([P, CT, C], FP8, tag=f"{nm}T8", name=f"{nm}T8")
              for nm in ("wq", "wk")}
        x_sb = persist.tile([P, CT, L], BF16, tag="x", name="x_sb")
        x8_sb = persist.tile([P, CT, L], FP8, tag="x8", name="x8_sb")
        k_sb = [persist.tile([P, L], BF16, tag=f"k{ot}", name=f"k{ot}")
                for ot in range(CT)]
        vt_sb = [persist.tile([P, H, D + 1], BF16, tag=f"vt{lt}", name=f"vt{lt}")
                 for lt in range(L // P)]
        # fp8 V for the off-diagonal kt pairs (kt < 12 only), interleaved by
        # kt parity for DoubleRow; 72-element stride keeps step%16==0
        vt8_sb = [persist.tile([P, 2, H, 72], FP8, tag=f"v8{pr}", name=f"v8{pr}")
                  for pr in range(6)]

        bo_sb = const.tile([P, CT], F32, tag="bo", name="bo_sb")
        onesH = const.tile([P, H], F32, tag="onesH", name="onesH")
        nc.vector.memset(onesH, 1.0)
        ones_f = const.tile([1, D], F32, tag="onesf", name="ones_f")
        nc.vector.memset(ones_f, 1.0)
        ones_r = const.tile([1, D], F32R, tag="onesr", name="ones_r")
        nc.vector.tensor_copy(ones_r, ones_f)

        # ---- input DMA: per-ct descriptors (each lands on its own hardware
        # DMA queue -> parallel transfers), posted from the three DMA-capable
        # engines. The ACT engine only gets a few early-critical ones (it is
        # the softmax pacemaker later); gpsimd posts a batch, then blocks on
        # its custom-op library load, then posts the non-urgent rest.
        wt_r = {nm: wt_d[nm].rearrange("(t p) o -> p t o", p=P) for nm in BF_NAMES}
        w8_r = {nm: w8_d[nm].rearrange("(t p) o -> p t o", p=P)
                for nm in ("wq", "wk")}
        x_r = x_d.rearrange("(t p) l -> p t l", p=P)
        x8_r = x8_d.rearrange("(t p) l -> p t l", p=P)

        def dreq(dst, src):
            return (dst, src)

        gp_early = [dreq(w8["wk"][:, ct, :], w8_r["wk"][:, ct, :])
                    for ct in (1, 2, 3)]
        gp_early += [dreq(x8_sb[:, ct, 0:QBLK], x8_r[:, ct, 0:QBLK])
                     for ct in (2, 3)]
        sc_early = [dreq(x8_sb[:, 1, 0:QBLK], x8_r[:, 1, 0:QBLK]),
                    dreq(w8["wq"][:, 1, :], w8_r["wq"][:, 1, :]),
                    dreq(wT["wv"][:, 1, :], wt_r["wv"][:, 1, :]),
                    dreq(wT["wv"][:, 3, :], wt_r["wv"][:, 3, :])]
        sy_all = [dreq(w8["wk"][:, 0, :], w8_r["wk"][:, 0, :]),
                  dreq(x8_sb[:, 0, 0:QBLK], x8_r[:, 0, 0:QBLK]),
                  dreq(w8["wq"][:, 0, :], w8_r["wq"][:, 0, :]),
                  dreq(w8["wq"][:, 2, :], w8_r["wq"][:, 2, :]),
                  dreq(w8["wq"][:, 3, :], w8_r["wq"][:, 3, :]),
                  dreq(wT["wv"][:, 0, :], wt_r["wv"][:, 0, :]),
                  dreq(wT["wv"][:, 2, :], wt_r["wv"][:, 2, :])]
        sy_all += [dreq(x_sb[:, ct, 0:QBLK], x_r[:, ct, 0:QBLK])
                   for ct in range(CT)]
        sy_all += [dreq(x8_sb[:, ct, QBLK:L], x8_r[:, ct, QBLK:L])
                   for ct in range(CT)]
        sy_all += [dreq(x_sb[:, ct, QBLK:L], x_r[:, ct, QBLK:L])
                   for ct in range(CT)]
        gp_late = [dreq(wT["wo"][:, ct, :], wt_r["wo"][:, ct, :])
                   for ct in range(CT)]
        gp_late.append(dreq(bo_sb, bo_d.rearrange("(t p) -> p t", p=P)))

        for dst, src in gp_early:
            nc.gpsimd.dma_start(dst, src)
        for dst, src in sc_early:
            nc.scalar.dma_start(dst, src)
        # gpsimd: force the custom-op library load now (affine_select +
        # partition_broadcast live in it; first use otherwise stalls ~8us)
        nc.gpsimd.affine_select(
            out=scr_g[:, 0:64], in_=scr_g[:, 0:64],
            compare_op=mybir.AluOpType.is_ge, fill=0.0,
            base=0, channel_multiplier=-1, pattern=[[1, 64]])
        nc.gpsimd.partition_broadcast(scr_g[:, 0:32], scr_g[0:1, 0:32])
        # scalar: pull the EXP table load forward
        nc.scalar.activation(scr_e, warm_sb[0:1, 0:16], EXP)
        for dst, src in sy_all:
            nc.sync.dma_start(dst, src)
        for dst, src in gp_late:
            nc.gpsimd.dma_start(dst, src)

        # tensor: ~12 throwaway matmuls get HAM past its 3.4us window so the
        # first real projections run at 2.4 GHz
        for i in range(12):
            wps = ps_proj.tile([P, QBLK], F32, tag="proj", name="warm_ps")
            nc.tensor.matmul(wps, lhsT=warm_sb[:, 0:P], rhs=warm_sb,
                             start=True, stop=True)

        q_pool = ctx.enter_context(tc.tile_pool(name="q", bufs=2))
        oc_pool = ctx.enter_context(tc.tile_pool(name="oc", bufs=2))
        pt_pool = ctx.enter_context(tc.tile_pool(name="pt", bufs=4))
        pt8_pool = ctx.enter_context(tc.tile_pool(name="pt8", bufs=3))
        nrm_pool = ctx.enter_context(tc.tile_pool(name="nrm", bufs=2))
        y_pool = ctx.enter_context(tc.tile_pool(name="y", bufs=2))

        # ---- projection helpers: each returns a list of unit thunks (one
        # PSUM group each) so filler work drips into the attention loop at
        # fine granularity. Q/K projections run fp8 DoubleRow (2 channel
        # tiles per matmul); V/Wo stay bf16.
        def qk_units(nm, ot, lc, fin_fn):
            def run():
                ps = ps_proj.tile([P, QBLK], F32, tag="proj", name="proj_ps")
                for cp in range(2):
                    nc.tensor.matmul(
                        ps, lhsT=w8[nm][:, 2 * cp:2 * cp + 2, ot * P:(ot + 1) * P],
                        rhs=x8_sb[:, 2 * cp:2 * cp + 2, lc * QBLK:(lc + 1) * QBLK],
                        start=(cp == 0), stop=(cp == 1), perf_mode=DR)
                fin_fn(ps)
            return [run]

        def proj_units(lhsT_of, rhs_of, fin_fn):
            def run():
                ps = ps_proj.tile([P, QBLK], F32, tag="proj", name="proj_ps")
                for ct in range(CT):
                    nc.tensor.matmul(
                        ps, lhsT=lhsT_of(ct), rhs=rhs_of(ct),
                        start=(ct == 0), stop=(ct == CT - 1))
                fin_fn(ps)
            return [run]

        def k_units(ot, lc):
            return qk_units(
                "wk", ot, lc,
                lambda ps: nc.vector.tensor_copy(
                    k_sb[ot][:, lc * QBLK:(lc + 1) * QBLK], ps))

        def v_units(lt):
            def fin(ps):
                t = vt_sb[lt]
                nc.vector.tensor_copy(t[:, :, D], onesH)
                nc.vector.tensor_copy(
                    t[:, :, 0:D], ps.rearrange("p (h d) -> p h d", d=D))
                if lt < 12:  # fp8 copy for the off-diagonal DoubleRow pairs
                    t8 = vt8_sb[lt // 2]
                    nc.vector.tensor_copy(t8[:, lt % 2, :, D], onesH)
                    nc.vector.tensor_copy(
                        t8[:, lt % 2, :, 0:D],
                        ps.rearrange("p (h d) -> p h d", d=D))
            return proj_units(
                lambda ct: x_sb[:, ct, lt * P:(lt + 1) * P],
                lambda ct: wT["wv"][:, ct, :], fin)

        q_tiles = {}

        def q_units(qt, ot):
            def fin(ps):
                nc.vector.tensor_copy(q_tiles[qt][:, ot, :], ps)
            units = qk_units("wq", ot, qt, fin)
            first = units[0]

            def f0():
                if qt not in q_tiles:
                    q_tiles[qt] = q_pool.tile([P, CT, QBLK], BF16, tag="q",
                                              name="q_sb")
                first()
            units[0] = f0
            return units

        oc_tiles = {}

        def wo_units(qt, ot):
            def fin(ps):
                ysb = y_pool.tile([P, QBLK], BF16, tag="y", name="y_sb")
                nc.vector.tensor_tensor(
                    ysb, ps, bo_sb[:, ot:ot + 1].to_broadcast((P, QBLK)),
                    mybir.AluOpType.add)
                nc.sync.dma_start(y_r[ot][:, qt * QBLK:(qt + 1) * QBLK], ysb)
            return proj_units(
                lambda ct: wT["wo"][:, ct, ot * P:(ot + 1) * P],
                lambda ct: oc_tiles[qt][ct], fin)

        def run_units(units):
            for u in units:
                u()

        # ---- warmup: the minimum for (qt0, hp0) to start
        run_units(k_units(0, 0))
        run_units(q_units(0, 0))
        run_units(v_units(0))

        # ---- filler queue: remaining projection work in need-order, drained
        # into the attention loop as PE filler.
        unit_q = []
        done = {("k", 0, 0), ("q", 0, 0), ("v", 0)}

        def enq(fid, units):
            for u in units[:-1]:
                unit_q.append((None, u))
            unit_q.append((fid, units[-1]))

        for lt in (1, 2, 3):
            enq(("v", lt), v_units(lt))
        for ot in (1, 2, 3):
            enq(("k", ot, 0), k_units(ot, 0))
            enq(("q", 0, ot), q_units(0, ot))
        for qt in (1, 2, 3):
            enq(("k", 0, qt), k_units(0, qt))
            enq(("q", qt, 0), q_units(qt, 0))
            for lt in range(4 * qt, 4 * qt + 4):
                enq(("v", lt), v_units(lt))
            for ot in (1, 2, 3):
                enq(("k", ot, qt), k_units(ot, qt))
                enq(("q", qt, ot), q_units(qt, ot))

        # Wo(qt-1) is reserved for qt's last head-pair so the PE stays at full
        # clock right up to the output tail
        wo_qs = {qt: [u for ot in range(CT) for u in wo_units(qt - 1, ot)]
                 for qt in (1, 2, 3)}

        def need(fid):
            if fid in done:
                return
            while unit_q:
                i, fn = unit_q.pop(0)
                fn()
                if i is not None:
                    done.add(i)
                    if i == fid:
                        return

        FILL_PER_KT = 0.4  # closures per kt (~340ns of PE work per kt slot)
        fill_acc = [0.0]

        def drip(qt, hp, kt, nkt):
            wq = wo_qs.get(qt) if hp == 3 else None
            if wq and kt % max(nkt // 4, 1) == 1:
                wq.pop(0)()
                return
            fill_acc[0] += FILL_PER_KT
            if unit_q and fill_acc[0] >= 1.0:
                fill_acc[0] -= 1.0
                i, fn = unit_q.pop(0)
                fn()
                if i is not None:
                    done.add(i)


        # ---- attention
        pend_norm = [None]
        wo3_part = {}

        def run_pend_norm():
            if pend_norm[0] is not None:
                pend_norm[0]()
                pend_norm[0] = None

        for qt in range(NQT):
            oc_tiles[qt] = [oc_pool.tile([P, QBLK], BF16, tag=f"oc{j}",
                                         name=f"oc{j}") for j in range(CT)]
            oc = oc_tiles[qt]

            for hp in range(HP):
                need(("k", hp, qt))
                need(("q", qt, hp))
                q_sb = q_tiles[qt]
                nkt = 4 * qt + 4
                av = [ps_av.tile([65, QBLK], F32, tag="av", name="av_ps")
                      for _ in range(2)]
                pend_av = []
                cur8 = [None]
                # diag tiles first (their masks run while the off-diag bulk
                # streams), off-diag pairs last: the final AV then depends
                # only on exp, keeping gpsimd off the boundary critical path
                kt_order = list(range(4 * qt, nkt)) + list(range(0, 4 * qt))
                for ki, kt in enumerate(kt_order):
                    j = kt - 4 * qt          # >=0 -> diagonal block index
                    co = 0 if j < 0 else P * j
                    cols = QBLK - co
                    # head a's S^T in PSUM bank 0, head b's in bank 1 (two
                    # concurrent row-group matmuls must not share a bank)
                    stp = ps_st.tile([P, 2 * QBLK], F32, tag="st", name="st_ps")
                    for sub, ofs in ((0, 0), (1, QBLK)):
                        pofs = sub * D
                        nc.tensor.matmul(
                            stp[:, ofs:ofs + cols],
                            lhsT=k_sb[hp][pofs:pofs + D, kt * P:(kt + 1) * P],
                            rhs=q_sb[pofs:pofs + D, hp, co:QBLK],
                            start=True, stop=True)
                    sv = stp.rearrange("p (g c) -> p g c", c=QBLK)[:, :, 0:cols]
                    if j < 0:
                        # off-diagonal: exp straight to fp8, paired by kt
                        # parity for the DoubleRow AV
                        if kt % 2 == 0:
                            cur8[0] = pt8_pool.tile([P, 2, 2, QBLK], FP8,
                                                    tag="pt8", name="pt8")
                        pt8 = cur8[0]
                        nc.scalar.activation(pt8[:, kt % 2, :, :], sv, EXP,
                                             scale=SCALE)
                        if kt % 2 == 1:
                            def av_pair(pt8=pt8, pr=kt // 2):
                                need(("v", 2 * pr + 1))
                                for sub in range(2):
                                    nc.tensor.matmul(
                                        av[sub][:, 0:QBLK],
                                        lhsT=vt8_sb[pr][:, :, 2 * hp + sub,
                                                        0:D + 1],
                                        rhs=pt8[:, :, sub, :],
                                        start=False, stop=True,
                                        skip_group_check=True, perf_mode=DR)
                            pend_av.append(av_pair)
                    else:
                        pt = pt_pool.tile([P, 2 * QBLK], BF16, tag="pt",
                                          name="pt_sb")
                        pv = pt.rearrange("p (g c) -> p g c",
                                          c=QBLK)[:, :, 0:cols]
                        nc.scalar.activation(pv, sv, EXP, scale=SCALE)
                        # only the first 128 columns of a diagonal tile touch
                        # the mask boundary (later columns are all-keep), and
                        # one 2D-pattern op masks both head slices at once
                        sl = pt.rearrange("p (g c) -> p g c",
                                          c=QBLK)[:, :, 0:P]
                        nc.gpsimd.affine_select(
                            out=sl, in_=sl,
                            compare_op=mybir.AluOpType.is_ge, fill=0.0,
                            base=0, channel_multiplier=-1,
                            pattern=[[0, 2], [1, P]])

                        def av_diag(pt=pt, kt=kt, co=co, cols=cols):
                            need(("v", kt))
                            for sub, ofs in ((0, 0), (1, QBLK)):
                                nc.tensor.matmul(
                                    av[sub][:, co:QBLK],
                                    lhsT=vt_sb[kt][:, 2 * hp + sub, :],
                                    rhs=pt[:, ofs:ofs + cols],
                                    start=(kt == 4 * qt), stop=True,
                                    skip_group_check=True)
                        pend_av.append(av_diag)
                    if ki == 1:
                        run_pend_norm()
                    drip(qt, hp, ki, nkt)
                    # pre-run Wo(qt3) ct0-2 for the first two output blocks
                    # during the final head-pair (held-open PSUM groups)
                    if qt == 3 and hp == 3 and ki in (nkt - 2, nkt - 1):
                        pot = ki - (nkt - 2)
                        wps = ps_proj.tile([P, QBLK], F32, tag="proj",
                                           name="proj_ps")
                        for ct in range(3):
                            nc.tensor.matmul(
                                wps,
                                lhsT=wT["wo"][:, ct, pot * P:(pot + 1) * P],
                                rhs=oc[ct], start=(ct == 0), stop=False,
                                skip_group_check=True)
                        wo3_part[pot] = wps
                    while len(pend_av) > 1:
                        pend_av.pop(0)()
                for fn in pend_av:
                    fn()

                last = (qt == NQT - 1 and hp == HP - 1)
                avs = nrm_pool.tile([65, 2, QBLK], F32, tag="avs", name="avs")
                if last:
                    # ---- tail norm, laid out for minimum critical path:
                    # pre-accumulate Wo ct0-2 for output blocks 2/3 in the
                    # retired score banks (blocks 0/1 are already pre-run in
                    # the proj banks), broadcast the raw denominator with
                    # small fp32r matmuls on the now-idle PE, then take the
                    # reciprocal across 64 lanes instead of 1; head b goes
                    # first so its partition-shift DMA overlaps head a's mul.
                    for pot in (2, 3):
                        wps = ps_st.tile([P, QBLK], F32, tag="st",
                                         name="wo_ps")
                        for ct in range(3):
                            nc.tensor.matmul(
                                wps,
                                lhsT=wT["wo"][:, ct, pot * P:(pot + 1) * P],
                                rhs=oc[ct], start=(ct == 0), stop=False,
                                skip_group_check=True)
                        wo3_part[pot] = wps
                    den0r = nrm_pool.tile([1, 2, QBLK], F32R, tag="den0r",
                                          name="den0r")
                    bc_ps = [ps_av.tile([D, QBLK], F32, tag="av", name="bc_ps")
                             for _ in range(2)]
                    inv = [nrm_pool.tile([D, QBLK], F32, tag=f"inv{s}",
                                         name="inv") for s in range(2)]
                    for sub in (1, 0):
                        nc.vector.tensor_copy(den0r[:, sub, :],
                                              av[sub][64:65, :])
                        nc.tensor.matmul(bc_ps[sub], lhsT=ones_r,
                                         rhs=den0r[:, sub, :],
                                         start=True, stop=True)
                        nc.vector.reciprocal_approx_fast(inv[sub], bc_ps[sub])
                        nc.vector.tensor_copy(avs[:, sub, :], av[sub])
                        if sub == 1:
                            tmp = nrm_pool.tile([D, QBLK], BF16, tag="tmp",
                                                name="tmp")
                            nc.vector.tensor_mul(tmp, avs[0:D, 1, :], inv[1])
                            nc.sync.dma_start(oc[hp][D:P, :], tmp)
                        else:
                            nc.vector.tensor_mul(oc[hp][0:D, :],
                                                 avs[0:D, 0, :], inv[0])
                else:
                    # steady state: AV staging first frees the accumulators
                    # for the next head-pair ASAP
                    for sub in range(2):
                        nc.vector.tensor_copy(avs[:, sub, :], av[sub])
                    den0 = nrm_pool.tile([1, 2, QBLK], F32, tag="den0",
                                         name="den0")
                    nc.vector.tensor_copy(den0, avs[64:65, :, :])
                    nc.vector.reciprocal_approx_fast(den0, den0)

                    def norm_tail(hp=hp, avs=avs, den0=den0, oc=oc):
                        bc = nrm_pool.tile([D, 2, QBLK], F32, tag="bc",
                                           name="bc")
                        nc.gpsimd.partition_broadcast(bc, den0)
                        nc.vector.tensor_mul(
                            oc[hp][0:D, :], avs[0:D, 0, :], bc[:, 0, :])
                        tmp = nrm_pool.tile([D, QBLK], BF16, tag="tmp",
                                            name="tmp")
                        nc.vector.tensor_mul(tmp, avs[0:D, 1, :], bc[:, 1, :])
                        nc.sync.dma_start(oc[hp][D:P, :], tmp)
                    pend_norm[0] = norm_tail

            run_pend_norm()
            for u in wo_qs.pop(qt, []):
                u()

        # tail: drain leftover fillers and finish the last Wo projection; the
        # bias-add runs on the idle ACT engine (it reads PSUM directly)
        while unit_q:
            i, fn = unit_q.pop(0)
            fn()
        for ot in range(CT):
            wps = wo3_part[ot]
            nc.tensor.matmul(
                wps, lhsT=wT["wo"][:, 3, ot * P:(ot + 1) * P],
                rhs=oc_tiles[3][3], start=False, stop=True,
                skip_group_check=True)
            ysb = y_pool.tile([P, QBLK], BF16, tag="y", name="y_sb")
            nc.scalar.add(ysb, wps, bo_sb[:, ot:ot + 1])
            nc.sync.dma_start(y_r[ot][:, 3 * QBLK:4 * QBLK], ysb)


_CACHE = {}


def _get_program():
    if "nc" not in _CACHE:
        nc = bacc.Bacc("TRN2", target_bir_lowering=False, debug=False,
                       num_devices=N_CORES)
        _emit(nc)
        nc.compile()
        _CACHE["nc"] = nc
    return _CACHE["nc"]


def _run(inputs, trace=False, **kwargs):
    import ml_dtypes
    nc = _get_program()
    bf16 = ml_dtypes.bfloat16
    fp8 = ml_dtypes.float8_e4m3
    xf = np.ascontiguousarray(np.asarray(inputs["x"], dtype=np.float32))
    x = xf.astype(bf16)
    x8 = xf.astype(fp8)
    shared = {}
    for nm in BF_NAMES:
        shared[nm + "t"] = np.ascontiguousarray(
            np.asarray(inputs[nm], dtype=np.float32).T).astype(bf16)
    for nm in ("wq", "wk"):
        shared[nm + "t8"] = np.ascontiguousarray(
            np.asarray(inputs[nm], dtype=np.float32).T * W8_SCALE).astype(fp8)
    shared["bo"] = np.ascontiguousarray(np.asarray(inputs["bo"], dtype=np.float32))
    in_maps = [{"x": np.ascontiguousarray(x[i]),
                "x8": np.ascontiguousarray(x8[i]), **shared}
               for i in range(N_CORES)]
    res = run_bass_kernel_spmd(nc, in_maps, core_ids=list(range(N_CORES)),
                               trace=trace, **kwargs)
    y = np.stack([np.asarray(res.results[i]["y"]).astype(np.float32)
                  for i in range(N_CORES)], axis=0)
    return y, res


def kernel(x, Wq, Wk, Wv, Wo, bo):
    y, _ = _run({"x": x, "wq": Wq, "wk": Wk, "wv": Wv, "wo": Wo, "bo": bo})
    return y


# revision 37
# speedup vs baseline: 1.0607x; 1.0034x over previous
"""Trainium2 Bass kernel for nn_ConvAttention (N=8, C=512, L=2048, 8 heads, causal).

Sharding: data-parallel over the batch dim N=8 -> one batch per NeuronCore.

v4 = v3 scheduling + fp8 DoubleRow on the PE-bound matmuls:
- Q/K projections contract two 128-channel tiles per matmul in fp8e4m3
  (Wq/Wk are pre-scaled by 64 on the host to stay in fp8 normal range; the
  softmax scale absorbs the 1/4096)
- off-diagonal attention*V matmuls contract two key tiles per matmul in fp8
  (exp output and V are quantized to fp8; every affected query attends to
  >=512 keys so the quantization noise averages out well below tolerance)
- V projection, Wo projection and the QK^T scores stay bf16 (early queries
  attend to few keys, so V-path fp8 noise would not average out there)
"""

import numpy as np
from contextlib import ExitStack

try:
    import concourse.bass as bass
except ImportError:  # concourse is on PYTHONPATH in the target container
    import sys
    sys.path.insert(0, "/opt/trn_rl_repo")
    import concourse.bass as bass

import concourse.tile as tile
from concourse import bacc, mybir
from concourse.bass_utils import run_bass_kernel_spmd

F32 = mybir.dt.float32
F32R = mybir.dt.float32r
BF16 = mybir.dt.bfloat16
FP8 = mybir.dt.float8e4
EXP = mybir.ActivationFunctionType.Exp
DR = mybir.MatmulPerfMode.DoubleRow

N_CORES = 8
N, C, L = 8, 512, 2048
H = 8
D = C // H            # 64
P = 128
CT = C // P           # 4 channel tiles
QBLK = 512            # q tile (matmul free dim)
NQT = L // QBLK       # 4 q tiles
HP = H // 2           # 4 head pairs (one per 128-channel tile)
W8_SCALE = 64.0       # host multiplies Wq/Wk by this before fp8 quantization
SCALE = float(C) ** -0.5 / (W8_SCALE * W8_SCALE)

BF_NAMES = ("wv", "wo")


def _emit(nc):
    # host passes x in bf16+fp8, Wv/Wo pre-transposed in bf16, Wq/Wk
    # pre-transposed, pre-scaled and quantized to fp8.
    x_d = nc.dram_tensor("x", [C, L], BF16, kind="ExternalInput").ap()
    x8_d = nc.dram_tensor("x8", [C, L], FP8, kind="ExternalInput").ap()
    wt_d = {nm: nc.dram_tensor(nm + "t", [C, C], BF16, kind="ExternalInput").ap()
            for nm in BF_NAMES}
    w8_d = {nm: nc.dram_tensor(nm + "t8", [C, C], FP8, kind="ExternalInput").ap()
            for nm in ("wq", "wk")}
    bo_d = nc.dram_tensor("bo", [C], F32, kind="ExternalInput").ap()
    y_d = nc.dram_tensor("y", [C, L], BF16, kind="ExternalOutput").ap()
    y_r = y_d.rearrange("(t p) l -> t p l", p=P)

    with tile.TileContext(nc) as tc, ExitStack() as ctx:
        const = ctx.enter_context(tc.tile_pool(name="const", bufs=1))
        persist = ctx.enter_context(tc.tile_pool(name="persist", bufs=1))

        ps_proj = ctx.enter_context(tc.tile_pool(name="ps_proj", bufs=2, space="PSUM"))
        ps_st = ctx.enter_context(tc.tile_pool(name="ps_st", bufs=2, space="PSUM"))
        ps_av = ctx.enter_context(tc.tile_pool(name="ps_av", bufs=2, space="PSUM"))

        # ---- warmup scratch (memset first so sim sees initialized data)
        warm_sb = const.tile([P, QBLK], BF16, tag="warm", name="warm_sb")
        nc.vector.memset(warm_sb, 0.0)
        scr_g = const.tile([P, 64], BF16, tag="scrg", name="scr_g")
        nc.gpsimd.memset(scr_g, 0.0)
        scr_e = const.tile([1, 16], F32, tag="scre", name="scr_e")

        # ---- persistent SBUF tensors
        wT = {nm: persist.tile([P, CT, C], BF16, tag=f"{nm}T", name=f"{nm}T")
              for nm in BF_NAMES}
        w8 = {nm: persist.tile([P, CT, C], FP8, tag=f"{nm}T8", name=f"{nm}T8")
              for nm in ("wq", "wk")}
        x_sb = persist.tile([P, CT, L], BF16, tag="x", name="x_sb")
        x8_sb = persist.tile([P, CT, L], FP8, tag="x8", name="x8_sb")
        k_sb = [persist.tile([P, L], BF16, tag=f"k{ot}", name=f"k{ot}")
                for ot in range(CT)]
        vt_sb = [persist.tile([P, H, D + 1], BF16, tag=f"vt{lt}", name=f"vt{lt}")
                 for lt in range(L // P)]
        # fp8 V for the off-diagonal kt pairs (kt < 12 only), interleaved by
        # kt parity for DoubleRow; 72-element stride keeps step%16==0
        vt8_sb = [persist.tile([P, 2, H, 72], FP8, tag=f"v8{pr}", name=f"v8{pr}")
                  for pr in range(6)]

        bo_sb = const.tile([P, CT], F32, tag="bo", name="bo_sb")
        onesH = const.tile([P, H], F32, tag="onesH", name="onesH")
        nc.vector.memset(onesH, 1.0)
        ones_f = const.tile([1, D], F32, tag="onesf", name="ones_f")
        nc.vector.memset(ones_f, 1.0)
        ones_r = const.tile([1, D], F32R, tag="onesr", name="ones_r")
        nc.vector.tensor_copy(ones_r, ones_f)

        # ---- input DMA: per-ct descriptors (each lands on its own hardware
        # DMA queue -> parallel transfers), posted from the three DMA-capable
        # engines. The ACT engine only gets a few early-critical ones (it is
        # the softmax pacemaker later); gpsimd posts a batch, then blocks on
        # its custom-op library load, then posts the non-urgent rest.
        wt_r = {nm: wt_d[nm].rearrange("(t p) o -> p t o", p=P) for nm in BF_NAMES}
        w8_r = {nm: w8_d[nm].rearrange("(t p) o -> p t o", p=P)
                for nm in ("wq", "wk")}
        x_r = x_d.rearrange("(t p) l -> p t l", p=P)
        x8_r = x8_d.rearrange("(t p) l -> p t l", p=P)

        def dreq(dst, src):
            return (dst, src)

        gp_early = [dreq(w8["wk"][:, ct, :], w8_r["wk"][:, ct, :])
                    for ct in (1, 2, 3)]
        gp_early += [dreq(x8_sb[:, ct, 0:QBLK], x8_r[:, ct, 0:QBLK])
                     for ct in (2, 3)]
        sc_early = [dreq(x8_sb[:, 1, 0:QBLK], x8_r[:, 1, 0:QBLK]),
                    dreq(w8["wq"][:, 1, :], w8_r["wq"][:, 1, :]),
                    dreq(wT["wv"][:, 1, :], wt_r["wv"][:, 1, :]),
                    dreq(wT["wv"][:, 3, :], wt_r["wv"][:, 3, :])]
        sy_all = [dreq(w8["wk"][:, 0, :], w8_r["wk"][:, 0, :]),
                  dreq(x8_sb[:, 0, 0:QBLK], x8_r[:, 0, 0:QBLK]),
                  dreq(w8["wq"][:, 0, :], w8_r["wq"][:, 0, :]),
                  dreq(w8["wq"][:, 2, :], w8_r["wq"][:, 2, :]),
                  dreq(w8["wq"][:, 3, :], w8_r["wq"][:, 3, :]),
                  dreq(wT["wv"][:, 0, :], wt_r["wv"][:, 0, :]),
                  dreq(wT["wv"][:, 2, :], wt_r["wv"][:, 2, :])]
        sy_all += [dreq(x_sb[:, ct, 0:QBLK], x_r[:, ct, 0:QBLK])
                   for ct in range(CT)]
        sy_all += [dreq(x8_sb[:, ct, QBLK:L], x8_r[:, ct, QBLK:L])
                   for ct in range(CT)]
        sy_all += [dreq(x_sb[:, ct, QBLK:L], x_r[:, ct, QBLK:L])
                   for ct in range(CT)]
        gp_late = [dreq(wT["wo"][:, ct, :], wt_r["wo"][:, ct, :])
                   for ct in range(CT)]
        gp_late.append(dreq(bo_sb, bo_d.rearrange("(t p) -> p t", p=P)))

        for dst, src in gp_early:
            nc.gpsimd.dma_start(dst, src)
        for dst, src in sc_early:
            nc.scalar.dma_start(dst, src)
        # gpsimd: force the custom-op library load now (affine_select +
        # partition_broadcast live in it; first use otherwise stalls ~8us)
        nc.gpsimd.affine_select(
            out=scr_g[:, 0:64], in_=scr_g[:, 0:64],
            compare_op=mybir.AluOpType.is_ge, fill=0.0,
            base=0, channel_multiplier=-1, pattern=[[1, 64]])
        nc.gpsimd.partition_broadcast(scr_g[:, 0:32], scr_g[0:1, 0:32])
        # scalar: pull the EXP table load forward
        nc.scalar.activation(scr_e, warm_sb[0:1, 0:16], EXP)
        for dst, src in sy_all:
            nc.sync.dma_start(dst, src)
        for dst, src in gp_late:
            nc.gpsimd.dma_start(dst, src)

        # tensor: ~12 throwaway matmuls get HAM past its 3.4us window so the
        # first real projections run at 2.4 GHz
        for i in range(12):
            wps = ps_proj.tile([P, QBLK], F32, tag="proj", name="warm_ps")
            nc.tensor.matmul(wps, lhsT=warm_sb[:, 0:P], rhs=warm_sb,
                             start=True, stop=True)

        q_pool = ctx.enter_context(tc.tile_pool(name="q", bufs=2))
        oc_pool = ctx.enter_context(tc.tile_pool(name="oc", bufs=2))
        pt_pool = ctx.enter_context(tc.tile_pool(name="pt", bufs=4))
        pt8_pool = ctx.enter_context(tc.tile_pool(name="pt8", bufs=3))
        nrm_pool = ctx.enter_context(tc.tile_pool(name="nrm", bufs=2))
        y_pool = ctx.enter_context(tc.tile_pool(name="y", bufs=2))

        # ---- projection helpers: each returns a list of unit thunks (one
        # PSUM group each) so filler work drips into the attention loop at
        # fine granularity. Q/K projections run fp8 DoubleRow (2 channel
        # tiles per matmul); V/Wo stay bf16.
        def qk_units(nm, ot, lc, fin_fn):
            def run():
                ps = ps_proj.tile([P, QBLK], F32, tag="proj", name="proj_ps")
                for cp in range(2):
                    nc.tensor.matmul(
                        ps, lhsT=w8[nm][:, 2 * cp:2 * cp + 2, ot * P:(ot + 1) * P],
                        rhs=x8_sb[:, 2 * cp:2 * cp + 2, lc * QBLK:(lc + 1) * QBLK],
                        start=(cp == 0), stop=(cp == 1), perf_mode=DR)
                fin_fn(ps)
            return [run]

        def proj_units(lhsT_of, rhs_of, fin_fn):
            def run():
                ps = ps_proj.tile([P, QBLK], F32, tag="proj", name="proj_ps")
                for ct in range(CT):
                    nc.tensor.matmul(
                        ps, lhsT=lhsT_of(ct), rhs=rhs_of(ct),
                        start=(ct == 0), stop=(ct == CT - 1))
                fin_fn(ps)
            return [run]

        def k_units(ot, lc):
            return qk_units(
                "wk", ot, lc,
                lambda ps: nc.vector.tensor_copy(
                    k_sb[ot][:, lc * QBLK:(lc + 1) * QBLK], ps))

        def v_units(lt):
            def fin(ps):
                t = vt_sb[lt]
                nc.vector.tensor_copy(t[:, :, D], onesH)
                nc.vector.tensor_copy(
                    t[:, :, 0:D], ps.rearrange("p (h d) -> p h d", d=D))
                if lt < 12:  # fp8 copy for the off-diagonal DoubleRow pairs
                    t8 = vt8_sb[lt // 2]
                    nc.vector.tensor_copy(t8[:, lt % 2, :, D], onesH)
                    nc.vector.tensor_copy(
                        t8[:, lt % 2, :, 0:D],
                        ps.rearrange("p (h d) -> p h d", d=D))
            return proj_units(
                lambda ct: x_sb[:, ct, lt * P:(lt + 1) * P],
                lambda ct: wT["wv"][:, ct, :], fin)

        q_tiles = {}

        def q_units(qt, ot):
            def fin(ps):
                nc.vector.tensor_copy(q_tiles[qt][:, ot, :], ps)
            units = qk_units("wq", ot, qt, fin)
            first = units[0]

            def f0():
                if qt not in q_tiles:
                    q_tiles[qt] = q_pool.tile([P, CT, QBLK], BF16, tag="q",
                                              name="q_sb")
                first()
            units[0] = f0
            return units

        oc_tiles = {}

        def wo_units(qt, ot):
            def fin(ps):
                ysb = y_pool.tile([P, QBLK], BF16, tag="y", name="y_sb")
                nc.vector.tensor_tensor(
                    ysb, ps, bo_sb[:, ot:ot + 1].to_broadcast((P, QBLK)),
                    mybir.AluOpType.add)
                nc.sync.dma_start(y_r[ot][:, qt * QBLK:(qt + 1) * QBLK], ysb)
            return proj_units(
                lambda ct: wT["wo"][:, ct, ot * P:(ot + 1) * P],
                lambda ct: oc_tiles[qt][ct], fin)

        def run_units(units):
            for u in units:
                u()

        # ---- warmup: the minimum for (qt0, hp0) to start
        run_units(k_units(0, 0))
        run_units(q_units(0, 0))
        run_units(v_units(0))

        # ---- filler queue: remaining projection work in need-order, drained
        # into the attention loop as PE filler.
        unit_q = []
        done = {("k", 0, 0), ("q", 0, 0), ("v", 0)}

        def enq(fid, units):
            for u in units[:-1]:
                unit_q.append((None, u))
            unit_q.append((fid, units[-1]))

        for lt in (1, 2, 3):
            enq(("v", lt), v_units(lt))
        for ot in (1, 2, 3):
            enq(("k", ot, 0), k_units(ot, 0))
            enq(("q", 0, ot), q_units(0, ot))
        for qt in (1, 2, 3):
            enq(("k", 0, qt), k_units(0, qt))
            enq(("q", qt, 0), q_units(qt, 0))
            for lt in range(4 * qt, 4 * qt + 4):
                enq(("v", lt), v_units(lt))
            for ot in (1, 2, 3):
                enq(("k", ot, qt), k_units(ot, qt))
                enq(("q", qt, ot), q_units(qt, ot))

        # Wo(qt-1) is reserved for qt's last head-pair so the PE stays at full
        # clock right up to the output tail
        wo_qs = {qt: [u for ot in range(CT) for u in wo_units(qt - 1, ot)]
                 for qt in (1, 2, 3)}

        def need(fid):
            if fid in done:
                return
            while unit_q:
                i, fn = unit_q.pop(0)
                fn()
                if i is not None:
                    done.add(i)
                    if i == fid:
                        return

        FILL_PER_KT = 0.4  # closures per kt (~340ns of PE work per kt slot)
        fill_acc = [0.0]

        def drip(qt, hp, kt, nkt):
            wq = wo_qs.get(qt) if hp == 3 else None
            if wq and kt % max(nkt // 4, 1) == 1:
                wq.pop(0)()
                return
            fill_acc[0] += FILL_PER_KT
            if unit_q and fill_acc[0] >= 1.0:
                fill_acc[0] -= 1.0
                i, fn = unit_q.pop(0)
                fn()
                if i is not None:
                    done.add(i)


        # ---- attention
        pend_norm = [None]
        wo3_part = {}

        def run_pend_norm():
            if pend_norm[0] is not None:
                pend_norm[0]()
                pend_norm[0] = None

        for qt in range(NQT):
            oc_tiles[qt] = [oc_pool.tile([P, QBLK], BF16, tag=f"oc{j}",
                                         name=f"oc{j}") for j in range(CT)]
            oc = oc_tiles[qt]

            for hp in range(HP):
                need(("k", hp, qt))
                need(("q", qt, hp))
                q_sb = q_tiles[qt]
                nkt = 4 * qt + 4
                av = [ps_av.tile([65, QBLK], F32, tag="av", name="av_ps")
                      for _ in range(2)]
                pend_av = []
                cur8 = [None]
                # diag tiles first (their masks run while the off-diag bulk
                # streams), off-diag pairs last: the final AV then depends
                # only on exp, keeping gpsimd off the boundary critical path
                kt_order = list(range(4 * qt, nkt)) + list(range(0, 4 * qt))
                for ki, kt in enumerate(kt_order):
                    j = kt - 4 * qt          # >=0 -> diagonal block index
                    co = 0 if j < 0 else P * j
                    cols = QBLK - co
                    # head a's S^T in PSUM bank 0, head b's in bank 1 (two
                    # concurrent row-group matmuls must not share a bank)
                    stp = ps_st.tile([P, 2 * QBLK], F32, tag="st", name="st_ps")
                    for sub, ofs in ((0, 0), (1, QBLK)):
                        pofs = sub * D
                        nc.tensor.matmul(
                            stp[:, ofs:ofs + cols],
                            lhsT=k_sb[hp][pofs:pofs + D, kt * P:(kt + 1) * P],
                            rhs=q_sb[pofs:pofs + D, hp, co:QBLK],
                            start=True, stop=True)
                    sv = stp.rearrange("p (g c) -> p g c", c=QBLK)[:, :, 0:cols]
                    if j < 0:
                        # off-diagonal: exp straight to fp8, paired by kt
                        # parity for the DoubleRow AV
                        if kt % 2 == 0:
                            cur8[0] = pt8_pool.tile([P, 2, 2, QBLK], FP8,
                                                    tag="pt8", name="pt8")
                        pt8 = cur8[0]
                        nc.scalar.activation(pt8[:, kt % 2, :, :], sv, EXP,
                                             scale=SCALE)
                        if kt % 2 == 1:
                            def av_pair(pt8=pt8, pr=kt // 2):
                                need(("v", 2 * pr + 1))
                                for sub in range(2):
                                    nc.tensor.matmul(
                                        av[sub][:, 0:QBLK],
                                        lhsT=vt8_sb[pr][:, :, 2 * hp + sub,
                                                        0:D + 1],
                                        rhs=pt8[:, :, sub, :],
                                        start=False, stop=True,
                                        skip_group_check=True, perf_mode=DR)
                            pend_av.append(av_pair)
                    else:
                        pt = pt_pool.tile([P, 2 * QBLK], BF16, tag="pt",
                                          name="pt_sb")
                        pv = pt.rearrange("p (g c) -> p g c",
                                          c=QBLK)[:, :, 0:cols]
                        nc.scalar.activation(pv, sv, EXP, scale=SCALE)
                        # only the first 128 columns of a diagonal tile touch
                        # the mask boundary (later columns are all-keep), and
                        # one 2D-pattern op masks both head slices at once
                        sl = pt.rearrange("p (g c) -> p g c",
                                          c=QBLK)[:, :, 0:P]
                        nc.gpsimd.affine_select(
                            out=sl, in_=sl,
                            compare_op=mybir.AluOpType.is_ge, fill=0.0,
                            base=0, channel_multiplier=-1,
                            pattern=[[0, 2], [1, P]])

                        def av_diag(pt=pt, kt=kt, co=co, cols=cols):
                            need(("v", kt))
                            for sub, ofs in ((0, 0), (1, QBLK)):
                                nc.tensor.matmul(
                                    av[sub][:, co:QBLK],
                                    lhsT=vt_sb[kt][:, 2 * hp + sub, :],
                                    rhs=pt[:, ofs:ofs + cols],
                                    start=(kt == 4 * qt), stop=True,
                                    skip_group_check=True)
                        pend_av.append(av_diag)
                    if ki == 1:
                        run_pend_norm()
                    drip(qt, hp, ki, nkt)
                    # pre-run Wo(qt3) ct0-2 for the first two output blocks
                    # during the final head-pair (held-open PSUM groups)
                    if qt == 3 and hp == 3 and ki in (nkt - 2, nkt - 1):
                        pot = ki - (nkt - 2)
                        wps = ps_proj.tile([P, QBLK], F32, tag="proj",
                                           name="proj_ps")
                        for ct in range(3):
                            nc.tensor.matmul(
                                wps,
                                lhsT=wT["wo"][:, ct, pot * P:(pot + 1) * P],
                                rhs=oc[ct], start=(ct == 0), stop=False,
                                skip_group_check=True)
                        wo3_part[pot] = wps
                    while len(pend_av) > 1:
                        pend_av.pop(0)()
                for fn in pend_av:
                    fn()

                last = (qt == NQT - 1 and hp == HP - 1)
                avs = nrm_pool.tile([65, 2, QBLK], F32, tag="avs", name="avs")
                if last:
                    # ---- tail norm, laid out for minimum critical path:
                    # pre-accumulate Wo ct0-2 for output blocks 2/3 in the
                    # retired score banks (blocks 0/1 are already pre-run in
                    # the proj banks), broadcast the raw denominator with
                    # small fp32r matmuls on the now-idle PE, then take the
                    # reciprocal across 64 lanes instead of 1; head b goes
                    # first so its partition-shift DMA overlaps head a's mul.
                    for pot in (2, 3):
                        wps = ps_st.tile([P, QBLK], F32, tag="st",
                                         name="wo_ps")
                        for ct in range(3):
                            nc.tensor.matmul(
                                wps,
                                lhsT=wT["wo"][:, ct, pot * P:(pot + 1) * P],
                                rhs=oc[ct], start=(ct == 0), stop=False,
                                skip_group_check=True)
                        wo3_part[pot] = wps
                    den0r = nrm_pool.tile([1, 2, QBLK], F32R, tag="den0r",
                                          name="den0r")
                    bc_ps = [ps_av.tile([D, QBLK], F32, tag="av", name="bc_ps")
                             for _ in range(2)]
                    inv = [nrm_pool.tile([D, QBLK], F32, tag=f"inv{s}",
                                         name="inv") for s in range(2)]
                    for sub in (1, 0):
                        nc.vector.tensor_copy(den0r[:, sub, :],
                                              av[sub][64:65, :])
                        nc.tensor.matmul(bc_ps[sub], lhsT=ones_r,
                                         rhs=den0r[:, sub, :],
                                         start=True, stop=True)
                        nc.vector.reciprocal_approx_fast(inv[sub], bc_ps[sub])
                        nc.vector.tensor_copy(avs[:, sub, :], av[sub])
                        if sub == 1:
                            tmp = nrm_pool.tile([D, QBLK], BF16, tag="tmp",
                                                name="tmp")
                            nc.vector.tensor_mul(tmp, avs[0:D, 1, :], inv[1])
                            nc.sync.dma_start(oc[hp][D:P, :], tmp)
                        else:
                            nc.vector.tensor_mul(oc[hp][0:D, :],
                                                 avs[0:D, 0, :], inv[0])
                else:
                    # steady state: AV staging first frees the accumulators
                    # for the next head-pair ASAP
                    for sub in range(2):
                        nc.vector.tensor_copy(avs[:, sub, :], av[sub])
                    den0 = nrm_pool.tile([1, 2, QBLK], F32, tag="den0",
                                         name="den0")
                    nc.vector.tensor_copy(den0, avs[64:65, :, :])
                    nc.vector.reciprocal_approx_fast(den0, den0)

                    def norm_tail(hp=hp, avs=avs, den0=den0, oc=oc):
                        bc = nrm_pool.tile([D, 2, QBLK], F32, tag="bc",
                                           name="bc")
                        nc.gpsimd.partition_broadcast(bc, den0)
                        nc.vector.tensor_mul(
                            oc[hp][0:D, :], avs[0:D, 0, :], bc[:, 0, :])
                        tmp = nrm_pool.tile([D, QBLK], BF16, tag="tmp",
                                            name="tmp")
                        nc.vector.tensor_mul(tmp, avs[0:D, 1, :], bc[:, 1, :])
                        nc.sync.dma_start(oc[hp][D:P, :], tmp)
                    pend_norm[0] = norm_tail

            run_pend_norm()
            for u in wo_qs.pop(qt, []):
                u()

        # tail: drain leftover fillers and finish the last Wo projection; the
        # bias-add runs on the idle ACT engine (it reads PSUM directly)
        while unit_q:
            i, fn = unit_q.pop(0)
            fn()
        for ot in range(CT):
            wps = wo3_part[ot]
            nc.tensor.matmul(
                wps, lhsT=wT["wo"][:, 3, ot * P:(ot + 1) * P],
                rhs=oc_tiles[3][3], start=False, stop=True,
                skip_group_check=True)
            ysb = y_pool.tile([P, QBLK], BF16, tag="y", name="y_sb")
            nc.scalar.add(ysb, wps, bo_sb[:, ot:ot + 1])
            nc.sync.dma_start(y_r[ot][:, 3 * QBLK:4 * QBLK], ysb)


_CACHE = {}


def _get_program():
    if "nc" not in _CACHE:
        nc = bacc.Bacc("TRN2", target_bir_lowering=False, debug=False,
                       num_devices=N_CORES)
        _emit(nc)
        nc.compile()
        _CACHE["nc"] = nc
    return _CACHE["nc"]


def _run(inputs, trace=False, **kwargs):
    import ml_dtypes
    nc = _get_program()
    bf16 = ml_dtypes.bfloat16
    fp8 = ml_dtypes.float8_e4m3
    xf = np.ascontiguousarray(np.asarray(inputs["x"], dtype=np.float32))
    x = xf.astype(bf16)
    x8 = xf.astype(fp8)
    shared = {}
    for nm in BF_NAMES:
        shared[nm + "t"] = np.ascontiguousarray(
            np.asarray(inputs[nm], dtype=np.float32).T).astype(bf16)
    for nm in ("wq", "wk"):
        shared[nm + "t8"] = np.ascontiguousarray(
            np.asarray(inputs[nm], dtype=np.float32).T * W8_SCALE).astype(fp8)
    shared["bo"] = np.ascontiguousarray(np.asarray(inputs["bo"], dtype=np.float32))
    in_maps = [{"x": np.ascontiguousarray(x[i]),
                "x8": np.ascontiguousarray(x8[i]), **shared}
               for i in range(N_CORES)]
    res = run_bass_kernel_spmd(nc, in_maps, core_ids=list(range(N_CORES)),
                               trace=trace, **kwargs)
    y = np.stack([np.asarray(res.results[i]["y"]).astype(np.float32)
                  for i in range(N_CORES)], axis=0)
    return y, res


def kernel(x, Wq, Wk, Wv, Wo, bo):
    y, _ = _run({"x": x, "wq": Wq, "wk": Wk, "wv": Wv, "wo": Wo, "bo": bo})
    return y
